# revision 50
# baseline (speedup 1.0000x reference)
"""Trainium2 Bass kernel for nn_BartAttention_66786741453241 (8 NeuronCores).

Reference (bugs preserved): no softmax — raw attention scores are used for the
AV matmul, and q is scaled by dh**-0.5 with scores further divided by sqrt(dh),
net 1/dh. The whole computation is therefore LINEAR in V, so we reassociate
    (Q K^T / 64) V  ==  Q (K^T V) / 64
which collapses the [T,T] score matrices into per-head [64,64] K^T V matrices
(~32x fewer attention FLOPs, exact in infinite precision).

Sharding: tensor-parallel by (batch, head-group) — core i handles batch i//4
and heads 4*(i%4) .. 4*(i%4)+4 for ALL 2048 tokens of that batch:
  - fused k|v projection (concatenated weight slice) -> per-head K^T V is
    complete locally: NO collective anywhere,
  - block-diagonal pair tiles of V^T K feed M_j = blockdiag(KTV) @ WoT_pair,
    so the tail is one matmul family: partial out^T = sum_j M_j^T @ qT_j,
  - qT projection for its 4 heads (bias + the net 1/64 scaling folded in),
  - partial out^T (bf16) DMA'd out per core.
The host sums the 4 partials per batch and adds bo — that host-side reduce is
the unshard step for the out_proj input-dim sharding (the "all-reduce after
out_proj" of the standard tensor-parallel recipe).

Schedule (from perfetto iteration; PE stream is packed 216ns/MM warm with
<2us of gaps, [~8us, ~71.5us] of the ~78us NEFF window):
  - input supply is TRANSFER-paced, not trigger-paced: hs rides as 8 2D
    DMAs into ONE [128, 8, 2048] tile — all eight 1024-token FIRST halves
    before any second half — and wkv as 4 two-chunk 3D DMAs interleaved in
    consumption order on the Sync queue.  ~11us of trigger issue for
    ~14.7us of transfer.  wq/wo ride behind hs (needed only from the Q/M
    phases); the tiny bias tiles (bq, cbd) ride the Scalar DGE queue in
    parallel.
  - kv projection wave 0 (token groups 0..6) is emitted CHUNK-major across
    psA's 7 banks so the PE FIFO order matches the half-chunk arrival
    order; wave 1 (groups 7..15) is group-major over resident data.
  - k|v biases never touch the device hot path: bias enters V^T K as
    rank-1 terms sv*bk^T + bv*sk^T + T*bv*bk^T, host-computed from hs
    column sums into per-pair diagonal [64,64] blocks (cbd) and added in
    the single V^T K evacuation.  kv evacuations are plain copies
    alternating Vector/Scalar.
  - V^T K matmuls (N=128, LDWEIGHTS-bound as a phase) are emitted right
    behind each group's evacuation, so their weight loads hide under
    neighbouring N=512 matmuls.
  - PE warm-up: 7 dummy matmuls bridge queue-start to first-chunk arrival
    and engage the HAM clock un-throttle (2.4 GHz after ~3.4us sustained).
  - tail: the last out^T chunk drains as [0:1024] after tg1, [1024:1536]
    after tg2 (computed as parallel-evacuated N=256 half-groups), and a
    final [1536:2048] whose two half-group evacuations AND DMA trigger all
    sit program-ordered on the Scalar queue — no cross-engine semaphore
    hop after the last matmul.
  - all matmuls bf16 (fp32 PSUM accumulate); end-to-end relative error vs
    the f32 reference ~4.7e-3 (gate 2e-2).
Rejected after measurement: a Gram-matrix reformulation (G = hs^T hs) needs
an inter-core AllReduce, but a 0.5 MB 8-core AllReduce measures ~70us warm on
this rig — any collective dwarfs the savings, so the collective-free
(batch x head-group) sharding with host-side partial summation stands.
"""

import os
import sys
import types

import numpy as np
import ml_dtypes

import concourse.bacc as bacc
import concourse.mybir as mybir
import concourse.tile as tile
from concourse.bass_utils import run_bass_kernel_spmd

BF16 = mybir.dt.bfloat16
F32 = mybir.dt.float32
NPBF16 = ml_dtypes.bfloat16

E = 1024        # embed dim
H = 16          # heads
DH = 64         # head dim
B, T = 2, 2048
NC = 8          # cores
P = 128
KC = E // P     # 8 contraction chunks for the in-projections
HPC = 4         # heads per core
EH = HPC * DH   # 256: per-core q/k/v feature width
TG = T // 512   # 4 moving-dim groups of 512 tokens
TTC = T // P    # 16 token chunks per core
Ident = mybir.ActivationFunctionType.Identity
N_DUMMY = 8     # PE warm-up matmuls (N=512)


def _install_axon_profile_hook():
    """Make trace=True usable under axon: register the NTFF hook that the
    staged antenv lacks, and neuter artifact upload (no bucket here). Safe
    no-op when pieces are missing."""
    try:
        import concourse.bass_utils as bu
        bu.upload_artifacts = lambda tmpdir: "local://" + tmpdir
    except Exception:
        pass
    if "antenv.axon_hooks" in sys.modules:
        return
    hook = None
    try:
        from trn_agent_boot.trn_boot import _ntff_profile_via_ctypes
        so = "/opt/axon/libaxon_pjrt.so"
        if os.path.exists(so):
            hook = _ntff_profile_via_ctypes(so)
    except Exception:
        hook = None
    mod = types.ModuleType("antenv.axon_hooks")
    mod.get_axon_ntff_profile_hook = lambda: hook
    mod.set_axon_ntff_profile_hook = lambda h: None
    sys.modules["antenv.axon_hooks"] = mod


def build():
    """Build + compile the per-core SPMD graph (identical on all 8 cores)."""
    nc = bacc.Bacc("TRN2", target_bir_lowering=False, debug=False, num_devices=NC)

    hsT = nc.dram_tensor("hsT", [E, T], BF16, kind="ExternalInput")       # 4 MB
    wkvt = nc.dram_tensor("wkvt", [E, 2 * EH], BF16, kind="ExternalInput")  # 1 MB
    # wq/wo pre-packed on host into SBUF partition layout -> one contiguous
    # 2D DMA each (128 descriptors, ~0.6us trigger) instead of a 1024-
    # descriptor 3D rearrange (~2.4us trigger)
    wqp = nc.dram_tensor("wqp", [P, KC * EH], BF16, kind="ExternalInput")  # 0.5 MB
    wop = nc.dram_tensor("wop", [P, 2 * E], BF16, kind="ExternalInput")    # 0.5 MB
    # bq/64 per-partition columns; cbd = host-computed k|v-bias correction to
    # the block-diagonal V^T K pair tiles (bias enters V^T K as rank-1 terms
    # sv*bk^T + bv*sk^T + T*bv*bk^T, computable on host from column sums of
    # hs — so K/V are projected WITHOUT bias and evacs are plain copies)
    bqt = nc.dram_tensor("bqt", [P, 2], F32, kind="ExternalInput")
    cbd = nc.dram_tensor("cbd", [P, 2, P], BF16, kind="ExternalInput")
    outT = nc.dram_tensor("outT", [E, T], BF16, kind="ExternalOutput")

    with tile.TileContext(nc) as tc:
        with (
            tc.tile_pool(name="sb", bufs=1) as sb,
            tc.tile_pool(name="stg", bufs=3) as stg,
            tc.tile_pool(name="psA", bufs=7, space="PSUM") as psA,
            tc.tile_pool(name="psB", bufs=1, space="PSUM") as psB,
        ):
            # ---- PE warm-up: dummy matmuls on a memset tile keep the PE's
            # HAM activity window busy during the input-DMA wait so the real
            # stream starts at 2.4 GHz instead of ramping from 1.2.
            dum_w = sb.tile([P, P], BF16, tag="dum_w")
            nc.gpsimd.memset(dum_w[:], 0.0)
            dum_x = sb.tile([P, 512], BF16, tag="dum_x")
            nc.gpsimd.memset(dum_x[:], 0.0)
            dum_ps = psB.tile([P, 512], F32, tag="psB")
            for _ in range(N_DUMMY):
                nc.tensor.matmul(dum_ps[:], dum_w[:], dum_x[:], start=True, stop=True)
            dum_out = sb.tile([P, 4], BF16, tag="dum_out")
            nc.vector.tensor_copy(dum_out[:], dum_ps[:, 0:4])

            # ---- input loads, all on the Sync queue in consumption byte
            # order.  The kv bias rides as a 1 KB [1,512] row (broadcast
            # across partitions on-device via a K=1 outer-product matmul)
            # and bq/64 as a tiny [128,2] tile, so no 0.26 MB bias tile
            # pollutes the critical byte path.  hs is ONE [128, 8, 2048]
            # tile; its FIRST token halves (tokens 0:1024, enough to finish
            # kv groups 0..7) stream before any second half, so the PE
            # saturates with backlog instead of chasing chunk arrivals.
            hs_big = sb.tile([P, KC, T], BF16, tag="hs")
            wkv_big = sb.tile([P, KC, 2 * EH], BF16, tag="wkv")
            bq_sb = sb.tile([P, 2], F32, tag="bq")
            hs3 = hsT.ap().rearrange("(c p) t -> p c t", p=P)
            wkv3 = wkvt.ap().rearrange("(c p) n -> p c n", p=P)

            TH = T // 2  # 1024-token halves

            def d_hs_a(c):
                nc.sync.dma_start(hs_big[:, c, 0:TH], hs3[:, c, 0:TH])

            def d_hs_b(c0, c1):
                nc.sync.dma_start(hs_big[:, c0:c1, TH:T], hs3[:, c0:c1, TH:T])

            def d_wkv(c0, c1):
                nc.sync.dma_start(wkv_big[:, c0:c1, :], wkv3[:, c0:c1, :])

            # bias tiles ride the Scalar DGE queue: issue in parallel with
            # Sync's critical wkv/hs triggers, ~66 KB so no HBM contention.
            nc.scalar.dma_start(bq_sb[:], bqt[:, :])
            cbd_sb = sb.tile([P, 2, P], BF16, tag="cbd")
            nc.scalar.dma_start(cbd_sb[:], cbd[:, :, :])
            d_wkv(0, 1)   # chunk 0's weights alone: first septet starts
            d_hs_a(0)     # ~0.35us earlier than with a wkv pair up front
            d_wkv(1, 2)
            d_hs_a(1)
            d_wkv(2, 4)
            d_hs_a(2)
            d_wkv(4, 6)
            d_hs_a(3)
            d_hs_a(4)
            d_wkv(6, 8)
            d_hs_a(5)
            d_hs_a(6)
            d_hs_a(7)
            d_hs_b(0, 2)
            d_hs_b(2, 4)
            d_hs_b(4, 6)
            d_hs_b(6, 8)

            # wq/wo ride the Sync queue behind all hs bytes (host-packed ->
            # cheap 128-descriptor 2D triggers)
            wq_big = sb.tile([P, KC * EH], BF16, tag="wq")
            nc.sync.dma_start(wq_big[:], wqp[:, :])
            wo_sb = sb.tile([P, 2, E], BF16, tag="wo")
            nc.sync.dma_start(wo_sb[:].rearrange("p c e -> p (c e)"), wop[:, :])

            # ---- fused k|v projection: [128 tokens, k(4 heads)|v(4 heads)]
            # with the pair-packed V^T K matmuls emitted right behind each
            # group's evacuation (their LDWEIGHTS hide under N=512 matmuls).
            # vtk_ps takes the psB bank for the whole loop (free once the
            # dummies evacuate); kv groups rotate through psA's 7 banks.
            kv_sb = [
                sb.tile([P, 2 * EH], BF16, tag=f"kv{tt}", name=f"kv{tt}")
                for tt in range(TTC)
            ]
            # per-head K^T V (pair-packed): ONE MM per (pair, chunk) —
            # lhsT = [v_A|v_B] against rhs = [k_A|k_B]; both pairs'
            # [128,128] outputs share one PSUM bank (256 f32/partition).
            # (vtk_ps is allocated AFTER wave 0's evacuations so the psB
            # bank can host wave 0's 8th group in the meantime)
            vtk_ps = None

            def kv_evac(tt, ps):
                # plain copy (no bias — host folds it into cbd), alternating
                # engines so evacuations don't serialize on one engine
                if tt % 2 == 0:
                    nc.vector.tensor_copy(kv_sb[tt][:], ps[:])
                else:
                    nc.scalar.copy(kv_sb[tt][:], ps[:])

            def vtk_mm(tt):
                for j in range(HPC // 2):
                    # start=True clears has_written for the WHOLE bank, so
                    # only the very first matmul may carry it: pair 1's
                    # first write then lands on cleared bits -> overwrite,
                    # which is exactly the accumulation restart we want.
                    nc.tensor.matmul(
                        vtk_ps[:, j * P:(j + 1) * P],
                        kv_sb[tt][:, EH + 2 * j * DH:EH + (2 * j + 2) * DH],
                        kv_sb[tt][:, 2 * j * DH:(2 * j + 2) * DH],
                        start=(tt == 0 and j == 0),
                        stop=(tt == TTC - 1 and j == HPC // 2 - 1),
                    )

            # wave 0: token groups 0..7 (= exactly the hs FIRST halves)
            # CHUNK-major across all 8 PSUM banks (7 psA + the psB bank,
            # idle until V^T K starts), so the PE FIFO order matches the hs
            # half-chunk arrival order and a full 1.7us of work unlocks per
            # arriving half-chunk — deep backlog against supply jitter.
            NW0 = 8
            ps_w = [
                psA.tile([P, 512], F32, tag="psA", name=f"kvps{tt}")
                for tt in range(NW0 - 1)
            ]
            ps_w.append(psB.tile([P, 512], F32, tag="psB", name="kvps7"))
            for c in range(KC):
                for tt in range(NW0):
                    nc.tensor.matmul(
                        ps_w[tt][:],
                        hs_big[:, c, tt * P:(tt + 1) * P],
                        wkv_big[:, c, :],
                        start=(c == 0),
                        stop=(c == KC - 1),
                    )
            for tt in range(NW0):
                kv_evac(tt, ps_w[tt])
            vtk_ps = psB.tile([P, 2 * P], F32, tag="psB")
            for tt in range(NW0):
                vtk_mm(tt)

            def kv_group(tt):
                ps = psA.tile([P, 512], F32, tag="psA", name=f"kvg{tt}")
                for c in range(KC):
                    nc.tensor.matmul(
                        ps[:],
                        hs_big[:, c, tt * P:(tt + 1) * P],
                        wkv_big[:, c, :],
                        start=(c == 0),
                        stop=(c == KC - 1),
                    )
                kv_evac(tt, ps)
                vtk_mm(tt)

            # wave 1: groups 8..13 CHUNK-major again — their tokens live in
            # the hs second halves, which arrive as chunk PAIRS while this
            # wave runs; chunk-major keeps the PE FIFO aligned with arrival
            # order (group-major would block the FIFO head on the last
            # b-pair while ready work for earlier chunks waits behind it)
            ps_w2 = [
                psA.tile([P, 512], F32, tag="psA", name=f"kvps2{tt}")
                for tt in range(8, 14)
            ]
            for c in range(KC):
                for i, tt in enumerate(range(8, 14)):
                    nc.tensor.matmul(
                        ps_w2[i][:],
                        hs_big[:, c, tt * P:(tt + 1) * P],
                        wkv_big[:, c, :],
                        start=(c == 0),
                        stop=(c == KC - 1),
                    )
            for i, tt in enumerate(range(8, 14)):
                kv_evac(tt, ps_w2[i])
                vtk_mm(tt)
            # groups 14..15 group-major over resident data
            kv_group(14)
            kv_group(15)

            # ---- block-diagonal V^T K pair tiles (useful diagonal [64,64]
            # blocks; VTK_h = KTV_h^T), then fold the out-projection:
            # M_j = blockdiag(KTV_A, KTV_B) @ WoT_pair
            vtk_bd = [
                sb.tile([P, P], BF16, tag=f"vtk_bd{j}", name=f"vtk_bd{j}")
                for j in range(HPC // 2)
            ]
            for j in range(HPC // 2):
                nc.gpsimd.memset(vtk_bd[j][:], 0.0)
            for j in range(HPC // 2):
                nc.vector.tensor_add(
                    vtk_bd[j][0:DH, 0:DH], vtk_ps[0:DH, j * P:j * P + DH],
                    cbd_sb[0:DH, j, 0:DH],
                )
                nc.vector.tensor_add(
                    vtk_bd[j][DH:2 * DH, DH:2 * DH],
                    vtk_ps[DH:2 * DH, j * P + DH:(j + 1) * P],
                    cbd_sb[DH:2 * DH, j, DH:2 * DH],
                )
            m_sb = [
                sb.tile([P, E], BF16, tag=f"m{j}", name=f"m{j}")
                for j in range(HPC // 2)
            ]
            for j in range(HPC // 2):
                for half in range(2):
                    ps = psB.tile([P, 512], F32, tag="psB")
                    nc.tensor.matmul(
                        ps[:],
                        vtk_bd[j][:],
                        wo_sb[:, j, half * 512:(half + 1) * 512],
                        start=True,
                        stop=True,
                    )
                    nc.vector.tensor_copy(
                        m_sb[j][:, half * 512:(half + 1) * 512], ps[:]
                    )

            # ---- qT projection [e_out 256, tokens], bias + 1/64 folded
            q_sb = [
                sb.tile([P, T], BF16, tag=f"q{m}", name=f"q{m}")
                for m in range(EH // P)
            ]
            for m in range(EH // P):
                for tg in range(TG):
                    ps = psA.tile([P, 512], F32, tag="psA")
                    for c in range(KC):
                        nc.tensor.matmul(
                            ps[:],
                            wq_big[:, c * EH + m * P:c * EH + (m + 1) * P],
                            hs_big[:, c, tg * 512:(tg + 1) * 512],
                            start=(c == 0),
                            stop=(c == KC - 1),
                        )
                    nc.scalar.activation(
                        q_sb[m][:, tg * 512:(tg + 1) * 512], ps[:], Ident,
                        bias=bq_sb[:, m:m + 1], scale=1.0 / 64.0,
                    )

            # ---- partial out^T = sum_j M_j^T @ qT_j (no bias: host adds bo)
            for m in range(KC):
                o_stage = stg.tile([P, T], BF16, tag="ostg")
                last = m == KC - 1
                for tg in range(TG):
                    if last and tg >= 2:
                        # last chunk's tg2+tg3 as N=256 half-groups in
                        # separate banks: Vector/Scalar evacuate in
                        # parallel, the DMA pieces chain early, and the
                        # very last piece is only 64 KB.
                        ph = [
                            psA.tile([P, 256], F32, tag="psA", name=f"ph{tg}{h}")
                            for h in range(2)
                        ]
                        for h in range(2):
                            for c in range(2):
                                nc.tensor.matmul(
                                    ph[h][:],
                                    m_sb[c][:, m * P:(m + 1) * P],
                                    q_sb[c][:, tg * 512 + h * 256:
                                             tg * 512 + (h + 1) * 256],
                                    start=(c == 0),
                                    stop=(c == 1),
                                )
                        lo = tg * 512
                        if tg == 2:
                            nc.vector.tensor_copy(
                                o_stage[:, lo:lo + 256], ph[0][:]
                            )
                            nc.scalar.copy(o_stage[:, lo + 256:lo + 512], ph[1][:])
                            nc.sync.dma_start(
                                outT[m * P:(m + 1) * P, 1024:1536],
                                o_stage[:, 1024:1536],
                            )
                        else:
                            # final 512 tokens: Vector (idle by now) takes
                            # the first half in parallel with Scalar's
                            # second; Scalar then triggers the single last
                            # DMA program-ordered behind its own copy, with
                            # only one cross-engine semaphore already long
                            # satisfied.
                            nc.vector.tensor_copy(o_stage[:, lo:lo + 256], ph[0][:])
                            nc.scalar.copy(o_stage[:, lo + 256:lo + 512], ph[1][:])
                            nc.scalar.dma_start(
                                outT[m * P:(m + 1) * P, 1536:T],
                                o_stage[:, 1536:T],
                            )
                        continue
                    ps = psA.tile([P, 512], F32, tag="psA")
                    for c in range(2):
                        nc.tensor.matmul(
                            ps[:],
                            m_sb[c][:, m * P:(m + 1) * P],
                            q_sb[c][:, tg * 512:(tg + 1) * 512],
                            start=(c == 0),
                            stop=(c == 1),
                        )
                    if tg % 2 == 0:
                        nc.vector.tensor_copy(o_stage[:, tg * 512:(tg + 1) * 512], ps[:])
                    else:
                        nc.scalar.copy(o_stage[:, tg * 512:(tg + 1) * 512], ps[:])
                    if last and tg == 1:
                        # drain the last chunk eagerly so only the tail
                        # pieces remain after the final evacuations
                        nc.sync.dma_start(
                            outT[m * P:(m + 1) * P, 0:1024], o_stage[:, 0:1024]
                        )
                if not last:
                    nc.sync.dma_start(outT[m * P:(m + 1) * P, :], o_stage[:])

    nc.compile()
    return nc


_NC_CACHE = None


def _get_nc():
    global _NC_CACHE
    if _NC_CACHE is None:
        _install_axon_profile_hook()
        _NC_CACHE = build()
    return _NC_CACHE


def cbd_tile(bk_sl, bv_sl, sk, sv):
    """Host-side k|v-bias correction to the block-diagonal V^T K pair tiles.
    V^T K = V0^T K0 + sv*bk^T + bv*sk^T + T*bv*bk^T   (per head), where
    sk/sv are the column sums of the UNbiased K0/V0 = Wk/Wv @ hs.sum(tokens).
    Layout [128, 2, 128]: pair j's heads (2j, 2j+1) on the two diagonal
    [64,64] blocks of [:, j, :]."""
    t = np.zeros((P, 2, P), np.float32)
    for j in range(HPC // 2):
        for hh in range(2):
            h = 2 * j + hh
            sl = slice(h * DH, (h + 1) * DH)
            r = slice(hh * DH, (hh + 1) * DH)
            c = np.outer(sv[sl], bk_sl[sl]) + np.outer(bv_sl[sl], sk[sl]) \
                + T * np.outer(bv_sl[sl], bk_sl[sl])
            t[r, j, r] = c
    return t.astype(NPBF16)


def make_in_maps(hidden_states, Wq, bq, Wk, bk, Wv, bv, Wo, bo):
    f32 = np.float32
    hs = np.asarray(hidden_states, f32)
    WqT = np.asarray(Wq, f32).T    # [e_in, e_out]
    WkT = np.asarray(Wk, f32).T
    WvT = np.asarray(Wv, f32).T
    WoT = np.asarray(Wo, f32).T
    bq64 = np.asarray(bq, f32) / 64.0
    bk = np.asarray(bk, f32)
    bv = np.asarray(bv, f32)

    hsT_b = [
        np.ascontiguousarray(hs[b].T).astype(NPBF16) for b in range(B)
    ]
    s_b = [hs[b].sum(axis=0) for b in range(B)]  # [1024] column sums per batch
    in_maps = []
    for i in range(NC):
        g, r = divmod(i, HPC)
        sl = slice(r * EH, (r + 1) * EH)
        wkvt = np.concatenate([WkT[:, sl], WvT[:, sl]], axis=1)
        sk = s_b[g] @ WkT[:, sl]   # colsum of unbiased K0, this core's heads
        sv = s_b[g] @ WvT[:, sl]
        bqt = np.ascontiguousarray(bq64[sl].reshape(2, P).T.astype(np.float32))
        # pack wq/wo into SBUF partition layout for single contiguous DMAs
        wqp = WqT[:, sl].reshape(KC, P, EH).transpose(1, 0, 2).reshape(P, KC * EH)
        wop = WoT[sl, :].reshape(2, P, E).transpose(1, 0, 2).reshape(P, 2 * E)
        in_maps.append({
            "hsT": hsT_b[g],
            "wkvt": np.ascontiguousarray(wkvt).astype(NPBF16),
            "wqp": np.ascontiguousarray(wqp).astype(NPBF16),
            "wop": np.ascontiguousarray(wop).astype(NPBF16),
            "bqt": bqt,
            "cbd": cbd_tile(bk[sl], bv[sl], sk, sv),
        })
    return in_maps


def run(inputs, trace=False, **kw):
    """Run on 8 NeuronCores; returns (full_output [B,T,E] f32, BassKernelResults)."""
    nc = _get_nc()
    in_maps = make_in_maps(**inputs)
    # rare transient NRT_EXEC_UNIT_UNRECOVERABLE (~5% of runs observed) —
    # retry up to 2x; the happy path and the compiled NEFF are unchanged
    last_err = None
    for _attempt in range(3):
        try:
            res = run_bass_kernel_spmd(
                nc, in_maps, list(range(NC)), trace=trace, **kw
            )
            break
        except Exception as e:
            last_err = e
    else:
        raise last_err
    bo = np.asarray(inputs["bo"], np.float32)
    out = np.empty((B, T, E), np.float32)
    for g in range(B):
        acc = res.results[g * HPC]["outT"].astype(np.float32)
        for r in range(1, HPC):
            acc = acc + res.results[g * HPC + r]["outT"].astype(np.float32)
        out[g] = acc.T + bo
    return out, res


def kernel(**inputs):
    out, _ = run(inputs, trace=False)
    return out


# revision 53
# speedup vs baseline: 1.1588x; 1.1588x over previous
"""Trainium2 Bass kernel for nn_BartAttention_66786741453241 (8 NeuronCores).

Reference (bugs preserved): no softmax — raw attention scores are used for the
AV matmul, and q is scaled by dh**-0.5 with scores further divided by sqrt(dh),
net 1/dh. The whole computation is therefore LINEAR in V, so we reassociate
    (Q K^T / 64) V  ==  Q (K^T V) / 64
which collapses the [T,T] score matrices into per-head [64,64] K^T V matrices
(~32x fewer attention FLOPs, exact in infinite precision).

Sharding: tensor-parallel by (batch, head-group) — core i handles batch i//4
and heads 4*(i%4) .. 4*(i%4)+4 for ALL 2048 tokens of that batch:
  - fused k|v projection (concatenated weight slice) -> per-head K^T V is
    complete locally: NO collective anywhere,
  - block-diagonal pair tiles of V^T K feed M_j = blockdiag(KTV) @ WoT_pair,
    so the tail is one matmul family: partial out^T = sum_j M_j^T @ qT_j,
  - qT projection for its 4 heads (bias + the net 1/64 scaling folded in),
  - partial out^T (bf16) DMA'd out per core.
The host sums the 4 partials per batch and adds bo — that host-side reduce is
the unshard step for the out_proj input-dim sharding (the "all-reduce after
out_proj" of the standard tensor-parallel recipe).

Schedule (from perfetto iteration; PE stream is packed 216ns/MM warm with
<2us of gaps, [~8us, ~71.5us] of the ~78us NEFF window):
  - input supply is TRANSFER-paced, not trigger-paced: hs rides as 8 2D
    DMAs into ONE [128, 8, 2048] tile — all eight 1024-token FIRST halves
    before any second half — and wkv as 4 two-chunk 3D DMAs interleaved in
    consumption order on the Sync queue.  ~11us of trigger issue for
    ~14.7us of transfer.  wq/wo ride behind hs (needed only from the Q/M
    phases); the tiny bias tiles (bq, cbd) ride the Scalar DGE queue in
    parallel.
  - kv projection wave 0 (token groups 0..6) is emitted CHUNK-major across
    psA's 7 banks so the PE FIFO order matches the half-chunk arrival
    order; wave 1 (groups 7..15) is group-major over resident data.
  - k|v biases never touch the device hot path: bias enters V^T K as
    rank-1 terms sv*bk^T + bv*sk^T + T*bv*bk^T, host-computed from hs
    column sums into per-pair diagonal [64,64] blocks (cbd) and added in
    the single V^T K evacuation.  kv evacuations are plain copies
    alternating Vector/Scalar.
  - V^T K matmuls (N=128, LDWEIGHTS-bound as a phase) are emitted right
    behind each group's evacuation, so their weight loads hide under
    neighbouring N=512 matmuls.
  - PE warm-up: 7 dummy matmuls bridge queue-start to first-chunk arrival
    and engage the HAM clock un-throttle (2.4 GHz after ~3.4us sustained).
  - tail: the last out^T chunk drains as [0:1024] after tg1, [1024:1536]
    after tg2 (computed as parallel-evacuated N=256 half-groups), and a
    final [1536:2048] whose two half-group evacuations AND DMA trigger all
    sit program-ordered on the Scalar queue — no cross-engine semaphore
    hop after the last matmul.
  - all matmuls bf16 (fp32 PSUM accumulate); end-to-end relative error vs
    the f32 reference ~4.7e-3 (gate 2e-2).
Rejected after measurement: a Gram-matrix reformulation (G = hs^T hs) needs
an inter-core AllReduce, but a 0.5 MB 8-core AllReduce measures ~70us warm on
this rig — any collective dwarfs the savings, so the collective-free
(batch x head-group) sharding with host-side partial summation stands.
"""

import os
import sys
import types

import numpy as np
import ml_dtypes

import concourse.bacc as bacc
import concourse.mybir as mybir
import concourse.tile as tile
from concourse.bass_utils import run_bass_kernel_spmd

BF16 = mybir.dt.bfloat16
F32 = mybir.dt.float32
NPBF16 = ml_dtypes.bfloat16

E = 1024        # embed dim
H = 16          # heads
DH = 64         # head dim
B, T = 2, 2048
NC = 8          # cores
P = 128
KC = E // P     # 8 contraction chunks for the in-projections
HPC = 4         # heads per core
EH = HPC * DH   # 256: per-core q/k/v feature width
TG = T // 512   # 4 moving-dim groups of 512 tokens
TTC = T // P    # 16 token chunks per core
Ident = mybir.ActivationFunctionType.Identity
N_DUMMY = 8     # PE warm-up matmuls (N=512)


def _install_axon_profile_hook():
    """Make trace=True usable under axon: register the NTFF hook that the
    staged antenv lacks, and neuter artifact upload (no bucket here). Safe
    no-op when pieces are missing."""
    try:
        import concourse.bass_utils as bu
        bu.upload_artifacts = lambda tmpdir: "local://" + tmpdir
    except Exception:
        pass
    if "antenv.axon_hooks" in sys.modules:
        return
    hook = None
    try:
        from trn_agent_boot.trn_boot import _ntff_profile_via_ctypes
        so = "/opt/axon/libaxon_pjrt.so"
        if os.path.exists(so):
            hook = _ntff_profile_via_ctypes(so)
    except Exception:
        hook = None
    mod = types.ModuleType("antenv.axon_hooks")
    mod.get_axon_ntff_profile_hook = lambda: hook
    mod.set_axon_ntff_profile_hook = lambda h: None
    sys.modules["antenv.axon_hooks"] = mod


def build():
    """Build + compile the per-core SPMD graph (identical on all 8 cores)."""
    nc = bacc.Bacc("TRN2", target_bir_lowering=False, debug=False, num_devices=NC)

    hsT = nc.dram_tensor("hsT", [E, T], BF16, kind="ExternalInput")       # 4 MB
    wkvt = nc.dram_tensor("wkvt", [E, 2 * EH], BF16, kind="ExternalInput")  # 1 MB
    # wq/wo pre-packed on host into SBUF partition layout -> one contiguous
    # 2D DMA each (128 descriptors, ~0.6us trigger) instead of a 1024-
    # descriptor 3D rearrange (~2.4us trigger)
    wqp = nc.dram_tensor("wqp", [P, KC * EH], BF16, kind="ExternalInput")  # 0.5 MB
    wop = nc.dram_tensor("wop", [P, 2 * E], BF16, kind="ExternalInput")    # 0.5 MB
    # bq/64 per-partition columns; cbd = host-computed k|v-bias correction to
    # the block-diagonal V^T K pair tiles (bias enters V^T K as rank-1 terms
    # sv*bk^T + bv*sk^T + T*bv*bk^T, computable on host from column sums of
    # hs — so K/V are projected WITHOUT bias and evacs are plain copies)
    bqt = nc.dram_tensor("bqt", [P, 2], F32, kind="ExternalInput")
    cbd = nc.dram_tensor("cbd", [P, 2, P], BF16, kind="ExternalInput")
    outT = nc.dram_tensor("outT", [E, T], BF16, kind="ExternalOutput")

    with tile.TileContext(nc) as tc:
        with (
            tc.tile_pool(name="sb", bufs=1) as sb,
            tc.tile_pool(name="stg", bufs=3) as stg,
            tc.tile_pool(name="psA", bufs=7, space="PSUM") as psA,
            tc.tile_pool(name="psB", bufs=1, space="PSUM") as psB,
        ):
            # ---- PE warm-up: dummy matmuls on a memset tile keep the PE's
            # HAM activity window busy during the input-DMA wait so the real
            # stream starts at 2.4 GHz instead of ramping from 1.2.
            dum_w = sb.tile([P, P], BF16, tag="dum_w")
            nc.gpsimd.memset(dum_w[:], 0.0)
            dum_x = sb.tile([P, 512], BF16, tag="dum_x")
            nc.gpsimd.memset(dum_x[:], 0.0)
            dum_ps = psB.tile([P, 512], F32, tag="psB")
            for _ in range(N_DUMMY):
                nc.tensor.matmul(dum_ps[:], dum_w[:], dum_x[:], start=True, stop=True)
            dum_out = sb.tile([P, 4], BF16, tag="dum_out")
            nc.vector.tensor_copy(dum_out[:], dum_ps[:, 0:4])

            # ---- input loads, all on the Sync queue in consumption byte
            # order.  The kv bias rides as a 1 KB [1,512] row (broadcast
            # across partitions on-device via a K=1 outer-product matmul)
            # and bq/64 as a tiny [128,2] tile, so no 0.26 MB bias tile
            # pollutes the critical byte path.  hs is ONE [128, 8, 2048]
            # tile; its FIRST token halves (tokens 0:1024, enough to finish
            # kv groups 0..7) stream before any second half, so the PE
            # saturates with backlog instead of chasing chunk arrivals.
            hs_big = sb.tile([P, KC, T], BF16, tag="hs")
            wkv_big = sb.tile([P, KC, 2 * EH], BF16, tag="wkv")
            bq_sb = sb.tile([P, 2], F32, tag="bq")
            hs3 = hsT.ap().rearrange("(c p) t -> p c t", p=P)
            wkv3 = wkvt.ap().rearrange("(c p) n -> p c n", p=P)

            TH = T // 2  # 1024-token halves

            def d_hs_a(c):
                nc.sync.dma_start(hs_big[:, c, 0:TH], hs3[:, c, 0:TH])

            def d_hs_b(c0, c1):
                nc.sync.dma_start(hs_big[:, c0:c1, TH:T], hs3[:, c0:c1, TH:T])

            def d_wkv(c0, c1):
                nc.sync.dma_start(wkv_big[:, c0:c1, :], wkv3[:, c0:c1, :])

            # bias tiles ride the Scalar DGE queue: issue in parallel with
            # Sync's critical wkv/hs triggers, ~66 KB so no HBM contention.
            nc.scalar.dma_start(bq_sb[:], bqt[:, :])
            cbd_sb = sb.tile([P, 2, P], BF16, tag="cbd")
            nc.scalar.dma_start(cbd_sb[:], cbd[:, :, :])
            d_wkv(0, 1)   # chunk 0's weights alone: first septet starts
            d_hs_a(0)     # ~0.35us earlier than with a wkv pair up front
            d_wkv(1, 2)
            d_hs_a(1)
            d_wkv(2, 4)
            d_hs_a(2)
            d_wkv(4, 6)
            d_hs_a(3)
            d_hs_a(4)
            d_wkv(6, 8)
            d_hs_a(5)
            d_hs_a(6)
            d_hs_a(7)
            d_hs_b(0, 2)
            d_hs_b(2, 4)
            d_hs_b(4, 6)
            d_hs_b(6, 8)

            # wq/wo ride the Sync queue behind all hs bytes (host-packed ->
            # cheap 128-descriptor 2D triggers)
            wq_big = sb.tile([P, KC * EH], BF16, tag="wq")
            nc.sync.dma_start(wq_big[:], wqp[:, :])
            wo_sb = sb.tile([P, 2, E], BF16, tag="wo")
            nc.sync.dma_start(wo_sb[:].rearrange("p c e -> p (c e)"), wop[:, :])

            # ---- fused k|v projection: [128 tokens, k(4 heads)|v(4 heads)]
            # with the pair-packed V^T K matmuls emitted right behind each
            # group's evacuation (their LDWEIGHTS hide under N=512 matmuls).
            # vtk_ps takes the psB bank for the whole loop (free once the
            # dummies evacuate); kv groups rotate through psA's 7 banks.
            kv_sb = [
                sb.tile([P, 2 * EH], BF16, tag=f"kv{tt}", name=f"kv{tt}")
                for tt in range(TTC)
            ]
            # per-head K^T V (pair-packed): ONE MM per (pair, chunk) —
            # lhsT = [v_A|v_B] against rhs = [k_A|k_B]; both pairs'
            # [128,128] outputs share one PSUM bank (256 f32/partition).
            vtk_ps = psB.tile([P, 2 * P], F32, tag="psB")

            def kv_evac(tt, ps):
                # plain copy (no bias — host folds it into cbd), alternating
                # engines so evacuations don't serialize on one engine
                if tt % 2 == 0:
                    nc.vector.tensor_copy(kv_sb[tt][:], ps[:])
                else:
                    nc.scalar.copy(kv_sb[tt][:], ps[:])

            def vtk_mm(tt):
                for j in range(HPC // 2):
                    # start=True clears has_written for the WHOLE bank, so
                    # only the very first matmul may carry it: pair 1's
                    # first write then lands on cleared bits -> overwrite,
                    # which is exactly the accumulation restart we want.
                    nc.tensor.matmul(
                        vtk_ps[:, j * P:(j + 1) * P],
                        kv_sb[tt][:, EH + 2 * j * DH:EH + (2 * j + 2) * DH],
                        kv_sb[tt][:, 2 * j * DH:(2 * j + 2) * DH],
                        start=(tt == 0 and j == 0),
                        stop=(tt == TTC - 1 and j == HPC // 2 - 1),
                    )

            # wave 0: token groups 0..6 CHUNK-major across psA's 7 banks, so
            # the PE FIFO order matches the hs half-chunk arrival order (a
            # token-group-major order would stall the FIFO head on chunk c
            # while ready work for earlier chunks sits queued behind it)
            NW0 = 7
            ps_w = [
                psA.tile([P, 512], F32, tag="psA", name=f"kvps{tt}")
                for tt in range(NW0)
            ]
            for c in range(KC):
                for tt in range(NW0):
                    nc.tensor.matmul(
                        ps_w[tt][:],
                        hs_big[:, c, tt * P:(tt + 1) * P],
                        wkv_big[:, c, :],
                        start=(c == 0),
                        stop=(c == KC - 1),
                    )
            for tt in range(NW0):
                kv_evac(tt, ps_w[tt])
                vtk_mm(tt)

            # group 7 (tokens 896:1024, still in the a-halves) group-major —
            # it bridges the a->b supply boundary
            def kv_group(tt):
                ps = psA.tile([P, 512], F32, tag="psA", name=f"kvg{tt}")
                for c in range(KC):
                    nc.tensor.matmul(
                        ps[:],
                        hs_big[:, c, tt * P:(tt + 1) * P],
                        wkv_big[:, c, :],
                        start=(c == 0),
                        stop=(c == KC - 1),
                    )
                kv_evac(tt, ps)
                vtk_mm(tt)

            kv_group(7)
            # wave 1: groups 8..13 CHUNK-major again — their tokens live in
            # the hs second halves, which arrive as chunk PAIRS while this
            # wave runs; chunk-major keeps the PE FIFO aligned with arrival
            # order (group-major would block the FIFO head on the last
            # b-pair while ready work for earlier chunks waits behind it)
            ps_w2 = [
                psA.tile([P, 512], F32, tag="psA", name=f"kvps2{tt}")
                for tt in range(8, 14)
            ]
            for c in range(KC):
                for i, tt in enumerate(range(8, 14)):
                    nc.tensor.matmul(
                        ps_w2[i][:],
                        hs_big[:, c, tt * P:(tt + 1) * P],
                        wkv_big[:, c, :],
                        start=(c == 0),
                        stop=(c == KC - 1),
                    )
            for i, tt in enumerate(range(8, 14)):
                kv_evac(tt, ps_w2[i])
                vtk_mm(tt)
            # groups 14..15 group-major over resident data
            kv_group(14)
            kv_group(15)

            # ---- block-diagonal V^T K pair tiles (useful diagonal [64,64]
            # blocks; VTK_h = KTV_h^T), then fold the out-projection:
            # M_j = blockdiag(KTV_A, KTV_B) @ WoT_pair
            vtk_bd = [
                sb.tile([P, P], BF16, tag=f"vtk_bd{j}", name=f"vtk_bd{j}")
                for j in range(HPC // 2)
            ]
            for j in range(HPC // 2):
                nc.gpsimd.memset(vtk_bd[j][:], 0.0)
            for j in range(HPC // 2):
                nc.vector.tensor_add(
                    vtk_bd[j][0:DH, 0:DH], vtk_ps[0:DH, j * P:j * P + DH],
                    cbd_sb[0:DH, j, 0:DH],
                )
                nc.vector.tensor_add(
                    vtk_bd[j][DH:2 * DH, DH:2 * DH],
                    vtk_ps[DH:2 * DH, j * P + DH:(j + 1) * P],
                    cbd_sb[DH:2 * DH, j, DH:2 * DH],
                )
            m_sb = [
                sb.tile([P, E], BF16, tag=f"m{j}", name=f"m{j}")
                for j in range(HPC // 2)
            ]
            for j in range(HPC // 2):
                for half in range(2):
                    ps = psB.tile([P, 512], F32, tag="psB")
                    nc.tensor.matmul(
                        ps[:],
                        vtk_bd[j][:],
                        wo_sb[:, j, half * 512:(half + 1) * 512],
                        start=True,
                        stop=True,
                    )
                    nc.vector.tensor_copy(
                        m_sb[j][:, half * 512:(half + 1) * 512], ps[:]
                    )

            # ---- qT projection [e_out 256, tokens], bias + 1/64 folded
            q_sb = [
                sb.tile([P, T], BF16, tag=f"q{m}", name=f"q{m}")
                for m in range(EH // P)
            ]
            for m in range(EH // P):
                for tg in range(TG):
                    ps = psA.tile([P, 512], F32, tag="psA")
                    for c in range(KC):
                        nc.tensor.matmul(
                            ps[:],
                            wq_big[:, c * EH + m * P:c * EH + (m + 1) * P],
                            hs_big[:, c, tg * 512:(tg + 1) * 512],
                            start=(c == 0),
                            stop=(c == KC - 1),
                        )
                    nc.scalar.activation(
                        q_sb[m][:, tg * 512:(tg + 1) * 512], ps[:], Ident,
                        bias=bq_sb[:, m:m + 1], scale=1.0 / 64.0,
                    )

            # ---- partial out^T = sum_j M_j^T @ qT_j (no bias: host adds bo)
            for m in range(KC):
                o_stage = stg.tile([P, T], BF16, tag="ostg")
                last = m == KC - 1
                for tg in range(TG):
                    if last and tg >= 2:
                        # last chunk's tg2+tg3 as N=256 half-groups in
                        # separate banks: Vector/Scalar evacuate in
                        # parallel, the DMA pieces chain early, and the
                        # very last piece is only 64 KB.
                        ph = [
                            psA.tile([P, 256], F32, tag="psA", name=f"ph{tg}{h}")
                            for h in range(2)
                        ]
                        for h in range(2):
                            for c in range(2):
                                nc.tensor.matmul(
                                    ph[h][:],
                                    m_sb[c][:, m * P:(m + 1) * P],
                                    q_sb[c][:, tg * 512 + h * 256:
                                             tg * 512 + (h + 1) * 256],
                                    start=(c == 0),
                                    stop=(c == 1),
                                )
                        lo = tg * 512
                        if tg == 2:
                            nc.vector.tensor_copy(
                                o_stage[:, lo:lo + 256], ph[0][:]
                            )
                            nc.scalar.copy(o_stage[:, lo + 256:lo + 512], ph[1][:])
                            nc.sync.dma_start(
                                outT[m * P:(m + 1) * P, 1024:1536],
                                o_stage[:, 1024:1536],
                            )
                        else:
                            # final 512 tokens: Vector (idle by now) takes
                            # the first half in parallel with Scalar's
                            # second; Scalar then triggers the single last
                            # DMA program-ordered behind its own copy, with
                            # only one cross-engine semaphore already long
                            # satisfied.
                            nc.vector.tensor_copy(o_stage[:, lo:lo + 256], ph[0][:])
                            nc.scalar.copy(o_stage[:, lo + 256:lo + 512], ph[1][:])
                            nc.scalar.dma_start(
                                outT[m * P:(m + 1) * P, 1536:T],
                                o_stage[:, 1536:T],
                            )
                        continue
                    ps = psA.tile([P, 512], F32, tag="psA")
                    for c in range(2):
                        nc.tensor.matmul(
                            ps[:],
                            m_sb[c][:, m * P:(m + 1) * P],
                            q_sb[c][:, tg * 512:(tg + 1) * 512],
                            start=(c == 0),
                            stop=(c == 1),
                        )
                    if tg % 2 == 0:
                        nc.vector.tensor_copy(o_stage[:, tg * 512:(tg + 1) * 512], ps[:])
                    else:
                        nc.scalar.copy(o_stage[:, tg * 512:(tg + 1) * 512], ps[:])
                    if last and tg == 1:
                        # drain the last chunk eagerly so only the tail
                        # pieces remain after the final evacuations
                        nc.sync.dma_start(
                            outT[m * P:(m + 1) * P, 0:1024], o_stage[:, 0:1024]
                        )
                if not last:
                    nc.sync.dma_start(outT[m * P:(m + 1) * P, :], o_stage[:])

    nc.compile()
    return nc


_NC_CACHE = None


def _get_nc():
    global _NC_CACHE
    if _NC_CACHE is None:
        _install_axon_profile_hook()
        _NC_CACHE = build()
    return _NC_CACHE


def cbd_tile(bk_sl, bv_sl, sk, sv):
    """Host-side k|v-bias correction to the block-diagonal V^T K pair tiles.
    V^T K = V0^T K0 + sv*bk^T + bv*sk^T + T*bv*bk^T   (per head), where
    sk/sv are the column sums of the UNbiased K0/V0 = Wk/Wv @ hs.sum(tokens).
    Layout [128, 2, 128]: pair j's heads (2j, 2j+1) on the two diagonal
    [64,64] blocks of [:, j, :]."""
    t = np.zeros((P, 2, P), np.float32)
    for j in range(HPC // 2):
        for hh in range(2):
            h = 2 * j + hh
            sl = slice(h * DH, (h + 1) * DH)
            r = slice(hh * DH, (hh + 1) * DH)
            c = np.outer(sv[sl], bk_sl[sl]) + np.outer(bv_sl[sl], sk[sl]) \
                + T * np.outer(bv_sl[sl], bk_sl[sl])
            t[r, j, r] = c
    return t.astype(NPBF16)


def make_in_maps(hidden_states, Wq, bq, Wk, bk, Wv, bv, Wo, bo):
    f32 = np.float32
    hs = np.asarray(hidden_states, f32)
    WqT = np.asarray(Wq, f32).T    # [e_in, e_out]
    WkT = np.asarray(Wk, f32).T
    WvT = np.asarray(Wv, f32).T
    WoT = np.asarray(Wo, f32).T
    bq64 = np.asarray(bq, f32) / 64.0
    bk = np.asarray(bk, f32)
    bv = np.asarray(bv, f32)

    hsT_b = [
        np.ascontiguousarray(hs[b].T).astype(NPBF16) for b in range(B)
    ]
    s_b = [hs[b].sum(axis=0) for b in range(B)]  # [1024] column sums per batch
    in_maps = []
    for i in range(NC):
        g, r = divmod(i, HPC)
        sl = slice(r * EH, (r + 1) * EH)
        wkvt = np.concatenate([WkT[:, sl], WvT[:, sl]], axis=1)
        sk = s_b[g] @ WkT[:, sl]   # colsum of unbiased K0, this core's heads
        sv = s_b[g] @ WvT[:, sl]
        bqt = np.ascontiguousarray(bq64[sl].reshape(2, P).T.astype(np.float32))
        # pack wq/wo into SBUF partition layout for single contiguous DMAs
        wqp = WqT[:, sl].reshape(KC, P, EH).transpose(1, 0, 2).reshape(P, KC * EH)
        wop = WoT[sl, :].reshape(2, P, E).transpose(1, 0, 2).reshape(P, 2 * E)
        in_maps.append({
            "hsT": hsT_b[g],
            "wkvt": np.ascontiguousarray(wkvt).astype(NPBF16),
            "wqp": np.ascontiguousarray(wqp).astype(NPBF16),
            "wop": np.ascontiguousarray(wop).astype(NPBF16),
            "bqt": bqt,
            "cbd": cbd_tile(bk[sl], bv[sl], sk, sv),
        })
    return in_maps


def run(inputs, trace=False, **kw):
    """Run on 8 NeuronCores; returns (full_output [B,T,E] f32, BassKernelResults)."""
    nc = _get_nc()
    in_maps = make_in_maps(**inputs)
    # rare transient NRT_EXEC_UNIT_UNRECOVERABLE (~5% of runs observed) —
    # retry up to 2x; the happy path and the compiled NEFF are unchanged
    last_err = None
    for _attempt in range(3):
        try:
            res = run_bass_kernel_spmd(
                nc, in_maps, list(range(NC)), trace=trace, **kw
            )
            break
        except Exception as e:
            last_err = e
    else:
        raise last_err
    bo = np.asarray(inputs["bo"], np.float32)
    out = np.empty((B, T, E), np.float32)
    for g in range(B):
        acc = res.results[g * HPC]["outT"].astype(np.float32)
        for r in range(1, HPC):
            acc = acc + res.results[g * HPC + r]["outT"].astype(np.float32)
        out[g] = acc.T + bo
    return out, res


def kernel(**inputs):
    out, _ = run(inputs, trace=False)
    return out


# revision 54
# speedup vs baseline: 1.1746x; 1.0136x over previous
"""Trainium2 Bass kernel for nn_BartAttention_66786741453241 (8 NeuronCores).

Reference (bugs preserved): no softmax — raw attention scores are used for the
AV matmul, and q is scaled by dh**-0.5 with scores further divided by sqrt(dh),
net 1/dh. The whole computation is therefore LINEAR in V, so we reassociate
    (Q K^T / 64) V  ==  Q (K^T V) / 64
which collapses the [T,T] score matrices into per-head [64,64] K^T V matrices
(~32x fewer attention FLOPs, exact in infinite precision).

Sharding: tensor-parallel by (batch, head-group) — core i handles batch i//4
and heads 4*(i%4) .. 4*(i%4)+4 for ALL 2048 tokens of that batch:
  - fused k|v projection (concatenated weight slice) -> per-head K^T V is
    complete locally: NO collective anywhere,
  - block-diagonal pair tiles of V^T K feed M_j = blockdiag(KTV) @ WoT_pair,
    so the tail is one matmul family: partial out^T = sum_j M_j^T @ qT_j,
  - qT projection for its 4 heads (bias + the net 1/64 scaling folded in),
  - partial out^T (bf16) DMA'd out per core.
The host sums the 4 partials per batch and adds bo — that host-side reduce is
the unshard step for the out_proj input-dim sharding (the "all-reduce after
out_proj" of the standard tensor-parallel recipe).

Schedule (from perfetto iteration; PE stream is packed 216ns/MM warm with
<2us of gaps, [~8us, ~71.5us] of the ~78us NEFF window):
  - input supply is TRANSFER-paced, not trigger-paced: hs rides as 8 2D
    DMAs into ONE [128, 8, 2048] tile — all eight 1024-token FIRST halves
    before any second half — and wkv as 4 two-chunk 3D DMAs interleaved in
    consumption order on the Sync queue.  ~11us of trigger issue for
    ~14.7us of transfer.  wq/wo ride behind hs (needed only from the Q/M
    phases); the tiny bias tiles (bq, cbd) ride the Scalar DGE queue in
    parallel.
  - kv projection waves are emitted CHUNK-major wherever supply is still
    streaming, so the strict PE FIFO order matches DMA arrival order:
    wave 0 = groups 0..6 across psA's 7 banks (hs first halves), then
    group 7 (a->b boundary), wave 1 = groups 8..13 chunk-major over the
    arriving second-half pairs, then groups 14..15 over resident data.
    (A group-major order stalls the FIFO head on a not-yet-arrived chunk
    while ready work sits queued behind it.)
  NOTE: sustained back-to-back benching drives the chip into the P0 power
  state (PE 2.4 -> ~2.0 GHz, kernel ~78us -> ~90us); it recovers after a
  few minutes idle.  The schedule is clock-invariant (gaps stay <2us).
  - k|v biases never touch the device hot path: bias enters V^T K as
    rank-1 terms sv*bk^T + bv*sk^T + T*bv*bk^T, host-computed from hs
    column sums into per-pair diagonal [64,64] blocks (cbd) and added in
    the single V^T K evacuation.  kv evacuations are plain copies
    alternating Vector/Scalar.
  - V^T K matmuls (N=128, LDWEIGHTS-bound as a phase) are emitted right
    behind each group's evacuation, so their weight loads hide under
    neighbouring N=512 matmuls.
  - PE warm-up: 7 dummy matmuls bridge queue-start to first-chunk arrival
    and engage the HAM clock un-throttle (2.4 GHz after ~3.4us sustained).
  - tail: the last out^T chunk drains as [0:1024] after tg1, [1024:1536]
    after tg2 (computed as parallel-evacuated N=256 half-groups), and a
    final [1536:2048] whose two half-group evacuations AND DMA trigger all
    sit program-ordered on the Scalar queue — no cross-engine semaphore
    hop after the last matmul.
  - all matmuls bf16 (fp32 PSUM accumulate); end-to-end relative error vs
    the f32 reference ~4.7e-3 (gate 2e-2).
Rejected after measurement: a Gram-matrix reformulation (G = hs^T hs) needs
an inter-core AllReduce, but a 0.5 MB 8-core AllReduce measures ~70us warm on
this rig — any collective dwarfs the savings, so the collective-free
(batch x head-group) sharding with host-side partial summation stands.
"""

import os
import sys
import types

import numpy as np
import ml_dtypes

import concourse.bacc as bacc
import concourse.mybir as mybir
import concourse.tile as tile
from concourse.bass_utils import run_bass_kernel_spmd

BF16 = mybir.dt.bfloat16
F32 = mybir.dt.float32
NPBF16 = ml_dtypes.bfloat16

E = 1024        # embed dim
H = 16          # heads
DH = 64         # head dim
B, T = 2, 2048
NC = 8          # cores
P = 128
KC = E // P     # 8 contraction chunks for the in-projections
HPC = 4         # heads per core
EH = HPC * DH   # 256: per-core q/k/v feature width
TG = T // 512   # 4 moving-dim groups of 512 tokens
TTC = T // P    # 16 token chunks per core
Ident = mybir.ActivationFunctionType.Identity
N_DUMMY = 8     # PE warm-up matmuls (N=512)


def _install_axon_profile_hook():
    """Make trace=True usable under axon: register the NTFF hook that the
    staged antenv lacks, and neuter artifact upload (no bucket here). Safe
    no-op when pieces are missing."""
    try:
        import concourse.bass_utils as bu
        bu.upload_artifacts = lambda tmpdir: "local://" + tmpdir
    except Exception:
        pass
    if "antenv.axon_hooks" in sys.modules:
        return
    hook = None
    try:
        from trn_agent_boot.trn_boot import _ntff_profile_via_ctypes
        so = "/opt/axon/libaxon_pjrt.so"
        if os.path.exists(so):
            hook = _ntff_profile_via_ctypes(so)
    except Exception:
        hook = None
    mod = types.ModuleType("antenv.axon_hooks")
    mod.get_axon_ntff_profile_hook = lambda: hook
    mod.set_axon_ntff_profile_hook = lambda h: None
    sys.modules["antenv.axon_hooks"] = mod


def build():
    """Build + compile the per-core SPMD graph (identical on all 8 cores)."""
    nc = bacc.Bacc("TRN2", target_bir_lowering=False, debug=False, num_devices=NC)

    hsT = nc.dram_tensor("hsT", [E, T], BF16, kind="ExternalInput")       # 4 MB
    wkvt = nc.dram_tensor("wkvt", [E, 2 * EH], BF16, kind="ExternalInput")  # 1 MB
    # wq/wo pre-packed on host into SBUF partition layout -> one contiguous
    # 2D DMA each (128 descriptors, ~0.6us trigger) instead of a 1024-
    # descriptor 3D rearrange (~2.4us trigger)
    wqp = nc.dram_tensor("wqp", [P, KC * EH], BF16, kind="ExternalInput")  # 0.5 MB
    wop = nc.dram_tensor("wop", [P, 2 * E], BF16, kind="ExternalInput")    # 0.5 MB
    # bq/64 per-partition columns; cbd = host-computed k|v-bias correction to
    # the block-diagonal V^T K pair tiles (bias enters V^T K as rank-1 terms
    # sv*bk^T + bv*sk^T + T*bv*bk^T, computable on host from column sums of
    # hs — so K/V are projected WITHOUT bias and evacs are plain copies)
    bqt = nc.dram_tensor("bqt", [P, 2], F32, kind="ExternalInput")
    cbd = nc.dram_tensor("cbd", [P, 2, P], BF16, kind="ExternalInput")
    outT = nc.dram_tensor("outT", [E, T], BF16, kind="ExternalOutput")

    with tile.TileContext(nc) as tc:
        with (
            tc.tile_pool(name="sb", bufs=1) as sb,
            tc.tile_pool(name="stg", bufs=3) as stg,
            tc.tile_pool(name="psA", bufs=7, space="PSUM") as psA,
            tc.tile_pool(name="psB", bufs=1, space="PSUM") as psB,
        ):
            # ---- PE warm-up: dummy matmuls on a memset tile keep the PE's
            # HAM activity window busy during the input-DMA wait so the real
            # stream starts at 2.4 GHz instead of ramping from 1.2.
            dum_w = sb.tile([P, P], BF16, tag="dum_w")
            nc.gpsimd.memset(dum_w[:], 0.0)
            dum_x = sb.tile([P, 512], BF16, tag="dum_x")
            nc.gpsimd.memset(dum_x[:], 0.0)
            dum_ps = psB.tile([P, 512], F32, tag="psB")
            for _ in range(N_DUMMY):
                nc.tensor.matmul(dum_ps[:], dum_w[:], dum_x[:], start=True, stop=True)
            dum_out = sb.tile([P, 4], BF16, tag="dum_out")
            nc.vector.tensor_copy(dum_out[:], dum_ps[:, 0:4])

            # ---- input loads, all on the Sync queue in consumption byte
            # order.  The kv bias rides as a 1 KB [1,512] row (broadcast
            # across partitions on-device via a K=1 outer-product matmul)
            # and bq/64 as a tiny [128,2] tile, so no 0.26 MB bias tile
            # pollutes the critical byte path.  hs is ONE [128, 8, 2048]
            # tile; its FIRST token halves (tokens 0:1024, enough to finish
            # kv groups 0..7) stream before any second half, so the PE
            # saturates with backlog instead of chasing chunk arrivals.
            hs_big = sb.tile([P, KC, T], BF16, tag="hs")
            wkv_big = sb.tile([P, KC, 2 * EH], BF16, tag="wkv")
            bq_sb = sb.tile([P, 2], F32, tag="bq")
            hs3 = hsT.ap().rearrange("(c p) t -> p c t", p=P)
            wkv3 = wkvt.ap().rearrange("(c p) n -> p c n", p=P)

            TH = T // 2  # 1024-token halves

            def d_hs_a(c):
                nc.sync.dma_start(hs_big[:, c, 0:TH], hs3[:, c, 0:TH])

            def d_hs_b(c0, c1):
                nc.sync.dma_start(hs_big[:, c0:c1, TH:T], hs3[:, c0:c1, TH:T])

            def d_wkv(c0, c1):
                nc.sync.dma_start(wkv_big[:, c0:c1, :], wkv3[:, c0:c1, :])

            # bias tiles ride the Scalar DGE queue: issue in parallel with
            # Sync's critical wkv/hs triggers, ~66 KB so no HBM contention.
            nc.scalar.dma_start(bq_sb[:], bqt[:, :])
            cbd_sb = sb.tile([P, 2, P], BF16, tag="cbd")
            nc.scalar.dma_start(cbd_sb[:], cbd[:, :, :])
            d_wkv(0, 1)   # chunk 0's weights alone: first septet starts
            d_hs_a(0)     # ~0.35us earlier than with a wkv pair up front
            d_wkv(1, 2)
            d_hs_a(1)
            d_wkv(2, 4)
            d_hs_a(2)
            d_wkv(4, 6)
            d_hs_a(3)
            d_hs_a(4)
            d_wkv(6, 8)
            d_hs_a(5)
            d_hs_a(6)
            d_hs_a(7)
            d_hs_b(0, 2)
            d_hs_b(2, 4)
            d_hs_b(4, 6)
            d_hs_b(6, 8)

            # wq/wo ride the Sync queue behind all hs bytes (host-packed ->
            # cheap 128-descriptor 2D triggers)
            wq_big = sb.tile([P, KC * EH], BF16, tag="wq")
            nc.sync.dma_start(wq_big[:], wqp[:, :])
            wo_sb = sb.tile([P, 2, E], BF16, tag="wo")
            nc.sync.dma_start(wo_sb[:].rearrange("p c e -> p (c e)"), wop[:, :])

            # ---- fused k|v projection: [128 tokens, k(4 heads)|v(4 heads)]
            # with the pair-packed V^T K matmuls emitted right behind each
            # group's evacuation (their LDWEIGHTS hide under N=512 matmuls).
            # vtk_ps takes the psB bank for the whole loop (free once the
            # dummies evacuate); kv groups rotate through psA's 7 banks.
            kv_sb = [
                sb.tile([P, 2 * EH], BF16, tag=f"kv{tt}", name=f"kv{tt}")
                for tt in range(TTC)
            ]
            # per-head K^T V (pair-packed): ONE MM per (pair, chunk) —
            # lhsT = [v_A|v_B] against rhs = [k_A|k_B]; both pairs'
            # [128,128] outputs share one PSUM bank (256 f32/partition).
            vtk_ps = psB.tile([P, 2 * P], F32, tag="psB")

            def kv_evac(tt, ps):
                # plain copy (no bias — host folds it into cbd), alternating
                # engines so evacuations don't serialize on one engine
                if tt % 2 == 0:
                    nc.vector.tensor_copy(kv_sb[tt][:], ps[:])
                else:
                    nc.scalar.copy(kv_sb[tt][:], ps[:])

            def vtk_mm(tt):
                for j in range(HPC // 2):
                    # start=True clears has_written for the WHOLE bank, so
                    # only the very first matmul may carry it: pair 1's
                    # first write then lands on cleared bits -> overwrite,
                    # which is exactly the accumulation restart we want.
                    nc.tensor.matmul(
                        vtk_ps[:, j * P:(j + 1) * P],
                        kv_sb[tt][:, EH + 2 * j * DH:EH + (2 * j + 2) * DH],
                        kv_sb[tt][:, 2 * j * DH:(2 * j + 2) * DH],
                        start=(tt == 0 and j == 0),
                        stop=(tt == TTC - 1 and j == HPC // 2 - 1),
                    )

            # wave 0: token groups 0..6 CHUNK-major across psA's 7 banks, so
            # the PE FIFO order matches the hs half-chunk arrival order (a
            # token-group-major order would stall the FIFO head on chunk c
            # while ready work for earlier chunks sits queued behind it)
            NW0 = 7
            ps_w = [
                psA.tile([P, 512], F32, tag="psA", name=f"kvps{tt}")
                for tt in range(NW0)
            ]
            for c in range(KC):
                for tt in range(NW0):
                    nc.tensor.matmul(
                        ps_w[tt][:],
                        hs_big[:, c, tt * P:(tt + 1) * P],
                        wkv_big[:, c, :],
                        start=(c == 0),
                        stop=(c == KC - 1),
                    )
            for tt in range(NW0):
                kv_evac(tt, ps_w[tt])
                vtk_mm(tt)

            # group 7 (tokens 896:1024, still in the a-halves) group-major —
            # it bridges the a->b supply boundary
            def kv_group(tt):
                ps = psA.tile([P, 512], F32, tag="psA", name=f"kvg{tt}")
                for c in range(KC):
                    nc.tensor.matmul(
                        ps[:],
                        hs_big[:, c, tt * P:(tt + 1) * P],
                        wkv_big[:, c, :],
                        start=(c == 0),
                        stop=(c == KC - 1),
                    )
                kv_evac(tt, ps)
                vtk_mm(tt)

            kv_group(7)
            # wave 1: groups 8..13 CHUNK-major again — their tokens live in
            # the hs second halves, which arrive as chunk PAIRS while this
            # wave runs; chunk-major keeps the PE FIFO aligned with arrival
            # order (group-major would block the FIFO head on the last
            # b-pair while ready work for earlier chunks waits behind it)
            ps_w2 = [
                psA.tile([P, 512], F32, tag="psA", name=f"kvps2{tt}")
                for tt in range(8, 14)
            ]
            for c in range(KC):
                for i, tt in enumerate(range(8, 14)):
                    nc.tensor.matmul(
                        ps_w2[i][:],
                        hs_big[:, c, tt * P:(tt + 1) * P],
                        wkv_big[:, c, :],
                        start=(c == 0),
                        stop=(c == KC - 1),
                    )
            for i, tt in enumerate(range(8, 14)):
                kv_evac(tt, ps_w2[i])
                vtk_mm(tt)
            # groups 14..15 group-major over resident data
            kv_group(14)
            kv_group(15)

            # ---- block-diagonal V^T K pair tiles (useful diagonal [64,64]
            # blocks; VTK_h = KTV_h^T), then fold the out-projection:
            # M_j = blockdiag(KTV_A, KTV_B) @ WoT_pair
            vtk_bd = [
                sb.tile([P, P], BF16, tag=f"vtk_bd{j}", name=f"vtk_bd{j}")
                for j in range(HPC // 2)
            ]
            for j in range(HPC // 2):
                nc.gpsimd.memset(vtk_bd[j][:], 0.0)
            for j in range(HPC // 2):
                nc.vector.tensor_add(
                    vtk_bd[j][0:DH, 0:DH], vtk_ps[0:DH, j * P:j * P + DH],
                    cbd_sb[0:DH, j, 0:DH],
                )
                nc.vector.tensor_add(
                    vtk_bd[j][DH:2 * DH, DH:2 * DH],
                    vtk_ps[DH:2 * DH, j * P + DH:(j + 1) * P],
                    cbd_sb[DH:2 * DH, j, DH:2 * DH],
                )
            m_sb = [
                sb.tile([P, E], BF16, tag=f"m{j}", name=f"m{j}")
                for j in range(HPC // 2)
            ]
            for j in range(HPC // 2):
                for half in range(2):
                    ps = psB.tile([P, 512], F32, tag="psB")
                    nc.tensor.matmul(
                        ps[:],
                        vtk_bd[j][:],
                        wo_sb[:, j, half * 512:(half + 1) * 512],
                        start=True,
                        stop=True,
                    )
                    nc.vector.tensor_copy(
                        m_sb[j][:, half * 512:(half + 1) * 512], ps[:]
                    )

            # ---- qT projection [e_out 256, tokens], bias + 1/64 folded
            q_sb = [
                sb.tile([P, T], BF16, tag=f"q{m}", name=f"q{m}")
                for m in range(EH // P)
            ]
            for m in range(EH // P):
                for tg in range(TG):
                    ps = psA.tile([P, 512], F32, tag="psA")
                    for c in range(KC):
                        nc.tensor.matmul(
                            ps[:],
                            wq_big[:, c * EH + m * P:c * EH + (m + 1) * P],
                            hs_big[:, c, tg * 512:(tg + 1) * 512],
                            start=(c == 0),
                            stop=(c == KC - 1),
                        )
                    nc.scalar.activation(
                        q_sb[m][:, tg * 512:(tg + 1) * 512], ps[:], Ident,
                        bias=bq_sb[:, m:m + 1], scale=1.0 / 64.0,
                    )

            # ---- partial out^T = sum_j M_j^T @ qT_j (no bias: host adds bo)
            for m in range(KC):
                o_stage = stg.tile([P, T], BF16, tag="ostg")
                last = m == KC - 1
                for tg in range(TG):
                    if last and tg >= 2:
                        # last chunk's tg2+tg3 as N=256 half-groups in
                        # separate banks: Vector/Scalar evacuate in
                        # parallel, the DMA pieces chain early, and the
                        # very last piece is only 64 KB.
                        ph = [
                            psA.tile([P, 256], F32, tag="psA", name=f"ph{tg}{h}")
                            for h in range(2)
                        ]
                        for h in range(2):
                            for c in range(2):
                                nc.tensor.matmul(
                                    ph[h][:],
                                    m_sb[c][:, m * P:(m + 1) * P],
                                    q_sb[c][:, tg * 512 + h * 256:
                                             tg * 512 + (h + 1) * 256],
                                    start=(c == 0),
                                    stop=(c == 1),
                                )
                        lo = tg * 512
                        if tg == 2:
                            nc.vector.tensor_copy(
                                o_stage[:, lo:lo + 256], ph[0][:]
                            )
                            nc.scalar.copy(o_stage[:, lo + 256:lo + 512], ph[1][:])
                            nc.sync.dma_start(
                                outT[m * P:(m + 1) * P, 1024:1536],
                                o_stage[:, 1024:1536],
                            )
                        else:
                            # final 512 tokens: Vector (idle by now) takes
                            # the first half in parallel with Scalar's
                            # second; Scalar then triggers the single last
                            # DMA program-ordered behind its own copy, with
                            # only one cross-engine semaphore already long
                            # satisfied.
                            nc.vector.tensor_copy(o_stage[:, lo:lo + 256], ph[0][:])
                            nc.scalar.copy(o_stage[:, lo + 256:lo + 512], ph[1][:])
                            nc.scalar.dma_start(
                                outT[m * P:(m + 1) * P, 1536:T],
                                o_stage[:, 1536:T],
                            )
                        continue
                    ps = psA.tile([P, 512], F32, tag="psA")
                    for c in range(2):
                        nc.tensor.matmul(
                            ps[:],
                            m_sb[c][:, m * P:(m + 1) * P],
                            q_sb[c][:, tg * 512:(tg + 1) * 512],
                            start=(c == 0),
                            stop=(c == 1),
                        )
                    if tg % 2 == 0:
                        nc.vector.tensor_copy(o_stage[:, tg * 512:(tg + 1) * 512], ps[:])
                    else:
                        nc.scalar.copy(o_stage[:, tg * 512:(tg + 1) * 512], ps[:])
                    if last and tg == 1:
                        # drain the last chunk eagerly so only the tail
                        # pieces remain after the final evacuations
                        nc.sync.dma_start(
                            outT[m * P:(m + 1) * P, 0:1024], o_stage[:, 0:1024]
                        )
                if not last:
                    nc.sync.dma_start(outT[m * P:(m + 1) * P, :], o_stage[:])

    nc.compile()
    return nc


_NC_CACHE = None


def _get_nc():
    global _NC_CACHE
    if _NC_CACHE is None:
        _install_axon_profile_hook()
        _NC_CACHE = build()
    return _NC_CACHE


def cbd_tile(bk_sl, bv_sl, sk, sv):
    """Host-side k|v-bias correction to the block-diagonal V^T K pair tiles.
    V^T K = V0^T K0 + sv*bk^T + bv*sk^T + T*bv*bk^T   (per head), where
    sk/sv are the column sums of the UNbiased K0/V0 = Wk/Wv @ hs.sum(tokens).
    Layout [128, 2, 128]: pair j's heads (2j, 2j+1) on the two diagonal
    [64,64] blocks of [:, j, :]."""
    t = np.zeros((P, 2, P), np.float32)
    for j in range(HPC // 2):
        for hh in range(2):
            h = 2 * j + hh
            sl = slice(h * DH, (h + 1) * DH)
            r = slice(hh * DH, (hh + 1) * DH)
            c = np.outer(sv[sl], bk_sl[sl]) + np.outer(bv_sl[sl], sk[sl]) \
                + T * np.outer(bv_sl[sl], bk_sl[sl])
            t[r, j, r] = c
    return t.astype(NPBF16)


def make_in_maps(hidden_states, Wq, bq, Wk, bk, Wv, bv, Wo, bo):
    f32 = np.float32
    hs = np.asarray(hidden_states, f32)
    WqT = np.asarray(Wq, f32).T    # [e_in, e_out]
    WkT = np.asarray(Wk, f32).T
    WvT = np.asarray(Wv, f32).T
    WoT = np.asarray(Wo, f32).T
    bq64 = np.asarray(bq, f32) / 64.0
    bk = np.asarray(bk, f32)
    bv = np.asarray(bv, f32)

    hsT_b = [
        np.ascontiguousarray(hs[b].T).astype(NPBF16) for b in range(B)
    ]
    s_b = [hs[b].sum(axis=0) for b in range(B)]  # [1024] column sums per batch
    in_maps = []
    for i in range(NC):
        g, r = divmod(i, HPC)
        sl = slice(r * EH, (r + 1) * EH)
        wkvt = np.concatenate([WkT[:, sl], WvT[:, sl]], axis=1)
        sk = s_b[g] @ WkT[:, sl]   # colsum of unbiased K0, this core's heads
        sv = s_b[g] @ WvT[:, sl]
        bqt = np.ascontiguousarray(bq64[sl].reshape(2, P).T.astype(np.float32))
        # pack wq/wo into SBUF partition layout for single contiguous DMAs
        wqp = WqT[:, sl].reshape(KC, P, EH).transpose(1, 0, 2).reshape(P, KC * EH)
        wop = WoT[sl, :].reshape(2, P, E).transpose(1, 0, 2).reshape(P, 2 * E)
        in_maps.append({
            "hsT": hsT_b[g],
            "wkvt": np.ascontiguousarray(wkvt).astype(NPBF16),
            "wqp": np.ascontiguousarray(wqp).astype(NPBF16),
            "wop": np.ascontiguousarray(wop).astype(NPBF16),
            "bqt": bqt,
            "cbd": cbd_tile(bk[sl], bv[sl], sk, sv),
        })
    return in_maps


def run(inputs, trace=False, **kw):
    """Run on 8 NeuronCores; returns (full_output [B,T,E] f32, BassKernelResults)."""
    nc = _get_nc()
    in_maps = make_in_maps(**inputs)
    # rare transient NRT_EXEC_UNIT_UNRECOVERABLE (~5% of runs observed) —
    # retry up to 2x; the happy path and the compiled NEFF are unchanged
    last_err = None
    for _attempt in range(3):
        try:
            res = run_bass_kernel_spmd(
                nc, in_maps, list(range(NC)), trace=trace, **kw
            )
            break
        except Exception as e:
            last_err = e
    else:
        raise last_err
    bo = np.asarray(inputs["bo"], np.float32)
    out = np.empty((B, T, E), np.float32)
    for g in range(B):
        acc = res.results[g * HPC]["outT"].astype(np.float32)
        for r in range(1, HPC):
            acc = acc + res.results[g * HPC + r]["outT"].astype(np.float32)
        out[g] = acc.T + bo
    return out, res


def kernel(**inputs):
    out, _ = run(inputs, trace=False)
    return out


# revision 59
# speedup vs baseline: 1.1797x; 1.0043x over previous
"""Trainium2 Bass kernel for nn_BartAttention_66786741453241 (8 NeuronCores).

Reference (bugs preserved): no softmax — raw attention scores are used for the
AV matmul, and q is scaled by dh**-0.5 with scores further divided by sqrt(dh),
net 1/dh. The whole computation is therefore LINEAR in V, so we reassociate
    (Q K^T / 64) V  ==  Q (K^T V) / 64
which collapses the [T,T] score matrices into per-head [64,64] K^T V matrices
(~32x fewer attention FLOPs, exact in infinite precision).

Sharding: tensor-parallel by (batch, head-group) — core i handles batch i//4
and heads 4*(i%4) .. 4*(i%4)+4 for ALL 2048 tokens of that batch:
  - fused k|v projection (concatenated weight slice) -> per-head K^T V is
    complete locally: NO collective anywhere,
  - block-diagonal pair tiles of V^T K feed M_j = blockdiag(KTV) @ WoT_pair,
    so the tail is one matmul family: partial out^T = sum_j M_j^T @ qT_j,
  - qT projection for its 4 heads (bias + the net 1/64 scaling folded in),
  - partial out^T (bf16) DMA'd out per core.
The host sums the 4 partials per batch and adds bo — that host-side reduce is
the unshard step for the out_proj input-dim sharding (the "all-reduce after
out_proj" of the standard tensor-parallel recipe).

Schedule (from perfetto iteration; PE stream is packed 216ns/MM warm with
<2us of gaps, [~8us, ~71.5us] of the ~78us NEFF window):
  - input supply is TRANSFER-paced, not trigger-paced: hs rides as 8 2D
    DMAs into ONE [128, 8, 2048] tile — all eight 1024-token FIRST halves
    before any second half — and wkv as 4 two-chunk 3D DMAs interleaved in
    consumption order on the Sync queue.  ~11us of trigger issue for
    ~14.7us of transfer.  wq/wo ride behind hs (needed only from the Q/M
    phases); the tiny bias tiles (bq, cbd) ride the Scalar DGE queue in
    parallel.
  - kv projection waves are emitted CHUNK-major wherever supply is still
    streaming, so the strict PE FIFO order matches DMA arrival order:
    wave 0 = groups 0..6 across psA's 7 banks (hs first halves), then
    group 7 (a->b boundary), wave 1 = groups 8..13 chunk-major over the
    arriving second-half pairs, then groups 14..15 over resident data.
    (A group-major order stalls the FIFO head on a not-yet-arrived chunk
    while ready work sits queued behind it.)
  NOTE: sustained back-to-back benching drives the chip into the P0 power
  state (PE 2.4 -> ~2.0 GHz, kernel ~78us -> ~90us); it recovers after a
  few minutes idle.  The schedule is clock-invariant (gaps stay <2us).
  - k|v biases never touch the device hot path: bias enters V^T K as
    rank-1 terms sv*bk^T + bv*sk^T + T*bv*bk^T, host-computed from hs
    column sums into per-pair diagonal [64,64] blocks (cbd) and added in
    the single V^T K evacuation.  kv evacuations are plain copies
    alternating Vector/Scalar.
  - V^T K matmuls (N=128, LDWEIGHTS-bound as a phase) are emitted right
    behind each group's evacuation, so their weight loads hide under
    neighbouring N=512 matmuls.
  - PE warm-up: 7 dummy matmuls bridge queue-start to first-chunk arrival
    and engage the HAM clock un-throttle (2.4 GHz after ~3.4us sustained).
  - tail: the last out^T chunk drains as [0:1024] after tg1, [1024:1536]
    after tg2 (computed as parallel-evacuated N=256 half-groups), and a
    final [1536:2048] whose two half-group evacuations AND DMA trigger all
    sit program-ordered on the Scalar queue — no cross-engine semaphore
    hop after the last matmul.
  - all matmuls bf16 (fp32 PSUM accumulate); end-to-end relative error vs
    the f32 reference ~4.7e-3 (gate 2e-2).
Rejected after measurement: a Gram-matrix reformulation (G = hs^T hs) needs
an inter-core AllReduce, but a 0.5 MB 8-core AllReduce measures ~70us warm on
this rig — any collective dwarfs the savings, so the collective-free
(batch x head-group) sharding with host-side partial summation stands.
"""

import os
import sys
import types

import numpy as np
import ml_dtypes

import concourse.bacc as bacc
import concourse.mybir as mybir
import concourse.tile as tile
from concourse.bass_utils import run_bass_kernel_spmd

BF16 = mybir.dt.bfloat16
F32 = mybir.dt.float32
NPBF16 = ml_dtypes.bfloat16

E = 1024        # embed dim
H = 16          # heads
DH = 64         # head dim
B, T = 2, 2048
NC = 8          # cores
P = 128
KC = E // P     # 8 contraction chunks for the in-projections
HPC = 4         # heads per core
EH = HPC * DH   # 256: per-core q/k/v feature width
TG = T // 512   # 4 moving-dim groups of 512 tokens
TTC = T // P    # 16 token chunks per core
Ident = mybir.ActivationFunctionType.Identity
N_DUMMY = 8     # PE warm-up matmuls (N=512)


def _install_axon_profile_hook():
    """Make trace=True usable under axon: register the NTFF hook that the
    staged antenv lacks, and neuter artifact upload (no bucket here). Safe
    no-op when pieces are missing."""
    try:
        import concourse.bass_utils as bu
        bu.upload_artifacts = lambda tmpdir: "local://" + tmpdir
    except Exception:
        pass
    if "antenv.axon_hooks" in sys.modules:
        return
    hook = None
    try:
        from trn_agent_boot.trn_boot import _ntff_profile_via_ctypes
        so = "/opt/axon/libaxon_pjrt.so"
        if os.path.exists(so):
            hook = _ntff_profile_via_ctypes(so)
    except Exception:
        hook = None
    mod = types.ModuleType("antenv.axon_hooks")
    mod.get_axon_ntff_profile_hook = lambda: hook
    mod.set_axon_ntff_profile_hook = lambda h: None
    sys.modules["antenv.axon_hooks"] = mod


def build():
    """Build + compile the per-core SPMD graph (identical on all 8 cores)."""
    nc = bacc.Bacc("TRN2", target_bir_lowering=False, debug=False, num_devices=NC)

    hsT = nc.dram_tensor("hsT", [E, T], BF16, kind="ExternalInput")       # 4 MB
    wkvt = nc.dram_tensor("wkvt", [E, 2 * EH], BF16, kind="ExternalInput")  # 1 MB
    # wq/wo pre-packed on host into SBUF partition layout -> one contiguous
    # 2D DMA each (128 descriptors, ~0.6us trigger) instead of a 1024-
    # descriptor 3D rearrange (~2.4us trigger)
    wqp = nc.dram_tensor("wqp", [P, KC * EH], BF16, kind="ExternalInput")  # 0.5 MB
    wop = nc.dram_tensor("wop", [P, 2 * E], BF16, kind="ExternalInput")    # 0.5 MB
    # wkv chunk 0 + hs chunk 0 first half share partition rows (E-rows
    # 0:128): host-fused into ONE tile so the first septet's data needs a
    # single trigger+transfer round-trip instead of two
    comb0 = nc.dram_tensor("comb0", [P, 2 * EH + T // 2], BF16, kind="ExternalInput")
    # bq/64 per-partition columns; cbd = host-computed k|v-bias correction to
    # the block-diagonal V^T K pair tiles (bias enters V^T K as rank-1 terms
    # sv*bk^T + bv*sk^T + T*bv*bk^T, computable on host from column sums of
    # hs — so K/V are projected WITHOUT bias and evacs are plain copies)
    bqt = nc.dram_tensor("bqt", [P, 2], F32, kind="ExternalInput")
    cbd = nc.dram_tensor("cbd", [P, 2, P], BF16, kind="ExternalInput")
    outT = nc.dram_tensor("outT", [E, T], BF16, kind="ExternalOutput")

    with tile.TileContext(nc) as tc:
        with (
            tc.tile_pool(name="sb", bufs=1) as sb,
            tc.tile_pool(name="stg", bufs=3) as stg,
            tc.tile_pool(name="psA", bufs=7, space="PSUM") as psA,
            tc.tile_pool(name="psB", bufs=1, space="PSUM") as psB,
        ):
            # ---- PE warm-up: dummy matmuls on a memset tile keep the PE's
            # HAM activity window busy during the input-DMA wait so the real
            # stream starts at 2.4 GHz instead of ramping from 1.2.
            dum_w = sb.tile([P, P], BF16, tag="dum_w")
            nc.gpsimd.memset(dum_w[:], 0.0)
            dum_x = sb.tile([P, 512], BF16, tag="dum_x")
            nc.gpsimd.memset(dum_x[:], 0.0)
            dum_ps = psB.tile([P, 512], F32, tag="psB")
            for _ in range(N_DUMMY):
                nc.tensor.matmul(dum_ps[:], dum_w[:], dum_x[:], start=True, stop=True)
            dum_out = sb.tile([P, 4], BF16, tag="dum_out")
            nc.vector.tensor_copy(dum_out[:], dum_ps[:, 0:4])

            # ---- input loads, all on the Sync queue in consumption byte
            # order.  The kv bias rides as a 1 KB [1,512] row (broadcast
            # across partitions on-device via a K=1 outer-product matmul)
            # and bq/64 as a tiny [128,2] tile, so no 0.26 MB bias tile
            # pollutes the critical byte path.  hs is ONE [128, 8, 2048]
            # tile; its FIRST token halves (tokens 0:1024, enough to finish
            # kv groups 0..7) stream before any second half, so the PE
            # saturates with backlog instead of chasing chunk arrivals.
            hs_big = sb.tile([P, KC, T], BF16, tag="hs")
            wkv_big = sb.tile([P, KC, 2 * EH], BF16, tag="wkv")
            bq_sb = sb.tile([P, 2], F32, tag="bq")
            hs3 = hsT.ap().rearrange("(c p) t -> p c t", p=P)
            wkv3 = wkvt.ap().rearrange("(c p) n -> p c n", p=P)

            TH = T // 2  # 1024-token halves
            comb0_sb = sb.tile([P, 2 * EH + TH], BF16, tag="comb0")

            def wkv_ap(c):
                return comb0_sb[:, 0:2 * EH] if c == 0 else wkv_big[:, c, :]

            def hs_ap(c, lo, hi):
                if c == 0 and hi <= TH:
                    return comb0_sb[:, 2 * EH + lo:2 * EH + hi]
                return hs_big[:, c, lo:hi]

            def d_hs_a(c):
                nc.sync.dma_start(hs_big[:, c, 0:TH], hs3[:, c, 0:TH])

            def d_hs_b(c0, c1):
                nc.sync.dma_start(hs_big[:, c0:c1, TH:T], hs3[:, c0:c1, TH:T])

            def d_wkv(c0, c1):
                nc.sync.dma_start(wkv_big[:, c0:c1, :], wkv3[:, c0:c1, :])

            # bias tiles ride the Scalar DGE queue: issue in parallel with
            # Sync's critical wkv/hs triggers, ~66 KB so no HBM contention.
            nc.scalar.dma_start(bq_sb[:], bqt[:, :])
            cbd_sb = sb.tile([P, 2, P], BF16, tag="cbd")
            nc.scalar.dma_start(cbd_sb[:], cbd[:, :, :])
            nc.sync.dma_start(comb0_sb[:], comb0[:, :])  # wkv0 + hs0 first half
            d_wkv(1, 2)
            d_hs_a(1)
            d_wkv(2, 4)
            d_hs_a(2)
            d_wkv(4, 6)
            d_hs_a(3)
            d_hs_a(4)
            d_wkv(6, 8)
            d_hs_a(5)
            d_hs_a(6)
            d_hs_a(7)
            d_hs_b(0, 2)
            d_hs_b(2, 4)
            d_hs_b(4, 6)
            d_hs_b(6, 8)

            # wq/wo ride the Sync queue behind all hs bytes (host-packed ->
            # cheap 128-descriptor 2D triggers)
            wq_big = sb.tile([P, KC * EH], BF16, tag="wq")
            nc.sync.dma_start(wq_big[:], wqp[:, :])
            wo_sb = sb.tile([P, 2, E], BF16, tag="wo")
            nc.sync.dma_start(wo_sb[:].rearrange("p c e -> p (c e)"), wop[:, :])

            # ---- fused k|v projection: [128 tokens, k(4 heads)|v(4 heads)]
            # with the pair-packed V^T K matmuls emitted right behind each
            # group's evacuation (their LDWEIGHTS hide under N=512 matmuls).
            # vtk_ps takes the psB bank for the whole loop (free once the
            # dummies evacuate); kv groups rotate through psA's 7 banks.
            kv_sb = [
                sb.tile([P, 2 * EH], BF16, tag=f"kv{tt}", name=f"kv{tt}")
                for tt in range(TTC)
            ]
            # per-head K^T V (pair-packed): ONE MM per (pair, chunk) —
            # lhsT = [v_A|v_B] against rhs = [k_A|k_B]; both pairs'
            # [128,128] outputs share one PSUM bank (256 f32/partition).
            vtk_ps = psB.tile([P, 2 * P], F32, tag="psB")

            def kv_evac(tt, ps):
                # plain copy (no bias — host folds it into cbd), alternating
                # engines so evacuations don't serialize on one engine
                if tt % 2 == 0:
                    nc.vector.tensor_copy(kv_sb[tt][:], ps[:])
                else:
                    nc.scalar.copy(kv_sb[tt][:], ps[:])

            def vtk_mm(tt):
                for j in range(HPC // 2):
                    # start=True clears has_written for the WHOLE bank, so
                    # only the very first matmul may carry it: pair 1's
                    # first write then lands on cleared bits -> overwrite,
                    # which is exactly the accumulation restart we want.
                    nc.tensor.matmul(
                        vtk_ps[:, j * P:(j + 1) * P],
                        kv_sb[tt][:, EH + 2 * j * DH:EH + (2 * j + 2) * DH],
                        kv_sb[tt][:, 2 * j * DH:(2 * j + 2) * DH],
                        start=(tt == 0 and j == 0),
                        stop=(tt == TTC - 1 and j == HPC // 2 - 1),
                    )

            # wave 0: token groups 0..6 CHUNK-major across psA's 7 banks, so
            # the PE FIFO order matches the hs half-chunk arrival order (a
            # token-group-major order would stall the FIFO head on chunk c
            # while ready work for earlier chunks sits queued behind it)
            NW0 = 7
            ps_w = [
                psA.tile([P, 512], F32, tag="psA", name=f"kvps{tt}")
                for tt in range(NW0)
            ]
            for c in range(KC):
                for tt in range(NW0):
                    nc.tensor.matmul(
                        ps_w[tt][:],
                        hs_ap(c, tt * P, (tt + 1) * P),
                        wkv_ap(c),
                        start=(c == 0),
                        stop=(c == KC - 1),
                    )
            for tt in range(NW0):
                kv_evac(tt, ps_w[tt])
                vtk_mm(tt)

            # group 7 (tokens 896:1024, still in the a-halves) group-major —
            # it bridges the a->b supply boundary
            def kv_group(tt):
                ps = psA.tile([P, 512], F32, tag="psA", name=f"kvg{tt}")
                for c in range(KC):
                    nc.tensor.matmul(
                        ps[:],
                        hs_ap(c, tt * P, (tt + 1) * P),
                        wkv_ap(c),
                        start=(c == 0),
                        stop=(c == KC - 1),
                    )
                kv_evac(tt, ps)
                vtk_mm(tt)

            kv_group(7)
            # wave 1: groups 8..13 CHUNK-major again — their tokens live in
            # the hs second halves, which arrive as chunk PAIRS while this
            # wave runs; chunk-major keeps the PE FIFO aligned with arrival
            # order (group-major would block the FIFO head on the last
            # b-pair while ready work for earlier chunks waits behind it)
            ps_w2 = [
                psA.tile([P, 512], F32, tag="psA", name=f"kvps2{tt}")
                for tt in range(8, 14)
            ]
            for c in range(KC):
                for i, tt in enumerate(range(8, 14)):
                    nc.tensor.matmul(
                        ps_w2[i][:],
                        hs_ap(c, tt * P, (tt + 1) * P),
                        wkv_ap(c),
                        start=(c == 0),
                        stop=(c == KC - 1),
                    )
            for i, tt in enumerate(range(8, 14)):
                kv_evac(tt, ps_w2[i])
                vtk_mm(tt)
            # groups 14..15 group-major over resident data
            kv_group(14)
            kv_group(15)

            # ---- block-diagonal V^T K pair tiles (useful diagonal [64,64]
            # blocks; VTK_h = KTV_h^T), then fold the out-projection:
            # M_j = blockdiag(KTV_A, KTV_B) @ WoT_pair
            vtk_bd = [
                sb.tile([P, P], BF16, tag=f"vtk_bd{j}", name=f"vtk_bd{j}")
                for j in range(HPC // 2)
            ]
            for j in range(HPC // 2):
                nc.gpsimd.memset(vtk_bd[j][:], 0.0)
            for j in range(HPC // 2):
                nc.vector.tensor_add(
                    vtk_bd[j][0:DH, 0:DH], vtk_ps[0:DH, j * P:j * P + DH],
                    cbd_sb[0:DH, j, 0:DH],
                )
                nc.vector.tensor_add(
                    vtk_bd[j][DH:2 * DH, DH:2 * DH],
                    vtk_ps[DH:2 * DH, j * P + DH:(j + 1) * P],
                    cbd_sb[DH:2 * DH, j, DH:2 * DH],
                )
            m_sb = [
                sb.tile([P, E], BF16, tag=f"m{j}", name=f"m{j}")
                for j in range(HPC // 2)
            ]
            for j in range(HPC // 2):
                for half in range(2):
                    ps = psB.tile([P, 512], F32, tag="psB")
                    nc.tensor.matmul(
                        ps[:],
                        vtk_bd[j][:],
                        wo_sb[:, j, half * 512:(half + 1) * 512],
                        start=True,
                        stop=True,
                    )
                    nc.vector.tensor_copy(
                        m_sb[j][:, half * 512:(half + 1) * 512], ps[:]
                    )

            # ---- qT projection [e_out 256, tokens], bias + 1/64 folded
            q_sb = [
                sb.tile([P, T], BF16, tag=f"q{m}", name=f"q{m}")
                for m in range(EH // P)
            ]
            for m in range(EH // P):
                for tg in range(TG):
                    ps = psA.tile([P, 512], F32, tag="psA")
                    for c in range(KC):
                        nc.tensor.matmul(
                            ps[:],
                            wq_big[:, c * EH + m * P:c * EH + (m + 1) * P],
                            hs_ap(c, tg * 512, (tg + 1) * 512),
                            start=(c == 0),
                            stop=(c == KC - 1),
                        )
                    nc.scalar.activation(
                        q_sb[m][:, tg * 512:(tg + 1) * 512], ps[:], Ident,
                        bias=bq_sb[:, m:m + 1], scale=1.0 / 64.0,
                    )

            # ---- partial out^T = sum_j M_j^T @ qT_j (no bias: host adds bo)
            for m in range(KC):
                o_stage = stg.tile([P, T], BF16, tag="ostg")
                last = m == KC - 1
                for tg in range(TG):
                    if last and tg >= 2:
                        # last chunk's tg2+tg3 as N=256 half-groups in
                        # separate banks: Vector/Scalar evacuate in
                        # parallel, the DMA pieces chain early, and the
                        # very last piece is only 64 KB.
                        ph = [
                            psA.tile([P, 256], F32, tag="psA", name=f"ph{tg}{h}")
                            for h in range(2)
                        ]
                        for h in range(2):
                            for c in range(2):
                                nc.tensor.matmul(
                                    ph[h][:],
                                    m_sb[c][:, m * P:(m + 1) * P],
                                    q_sb[c][:, tg * 512 + h * 256:
                                             tg * 512 + (h + 1) * 256],
                                    start=(c == 0),
                                    stop=(c == 1),
                                )
                        lo = tg * 512
                        if tg == 2:
                            nc.vector.tensor_copy(
                                o_stage[:, lo:lo + 256], ph[0][:]
                            )
                            nc.scalar.copy(o_stage[:, lo + 256:lo + 512], ph[1][:])
                            nc.sync.dma_start(
                                outT[m * P:(m + 1) * P, 1024:1536],
                                o_stage[:, 1024:1536],
                            )
                        else:
                            # final 512 tokens: Vector (idle by now) takes
                            # the first half in parallel with Scalar's
                            # second; Scalar then triggers the single last
                            # DMA program-ordered behind its own copy, with
                            # only one cross-engine semaphore already long
                            # satisfied.
                            nc.vector.tensor_copy(o_stage[:, lo:lo + 256], ph[0][:])
                            nc.scalar.copy(o_stage[:, lo + 256:lo + 512], ph[1][:])
                            nc.scalar.dma_start(
                                outT[m * P:(m + 1) * P, 1536:T],
                                o_stage[:, 1536:T],
                            )
                        continue
                    ps = psA.tile([P, 512], F32, tag="psA")
                    for c in range(2):
                        nc.tensor.matmul(
                            ps[:],
                            m_sb[c][:, m * P:(m + 1) * P],
                            q_sb[c][:, tg * 512:(tg + 1) * 512],
                            start=(c == 0),
                            stop=(c == 1),
                        )
                    if tg % 2 == 0:
                        nc.vector.tensor_copy(o_stage[:, tg * 512:(tg + 1) * 512], ps[:])
                    else:
                        nc.scalar.copy(o_stage[:, tg * 512:(tg + 1) * 512], ps[:])
                    if last and tg == 1:
                        # drain the last chunk eagerly so only the tail
                        # pieces remain after the final evacuations
                        nc.sync.dma_start(
                            outT[m * P:(m + 1) * P, 0:1024], o_stage[:, 0:1024]
                        )
                if not last:
                    nc.sync.dma_start(outT[m * P:(m + 1) * P, :], o_stage[:])

    nc.compile()
    return nc


_NC_CACHE = None


def _get_nc():
    global _NC_CACHE
    if _NC_CACHE is None:
        _install_axon_profile_hook()
        _NC_CACHE = build()
    return _NC_CACHE


def cbd_tile(bk_sl, bv_sl, sk, sv):
    """Host-side k|v-bias correction to the block-diagonal V^T K pair tiles.
    V^T K = V0^T K0 + sv*bk^T + bv*sk^T + T*bv*bk^T   (per head), where
    sk/sv are the column sums of the UNbiased K0/V0 = Wk/Wv @ hs.sum(tokens).
    Layout [128, 2, 128]: pair j's heads (2j, 2j+1) on the two diagonal
    [64,64] blocks of [:, j, :]."""
    t = np.zeros((P, 2, P), np.float32)
    for j in range(HPC // 2):
        for hh in range(2):
            h = 2 * j + hh
            sl = slice(h * DH, (h + 1) * DH)
            r = slice(hh * DH, (hh + 1) * DH)
            c = np.outer(sv[sl], bk_sl[sl]) + np.outer(bv_sl[sl], sk[sl]) \
                + T * np.outer(bv_sl[sl], bk_sl[sl])
            t[r, j, r] = c
    return t.astype(NPBF16)


def make_in_maps(hidden_states, Wq, bq, Wk, bk, Wv, bv, Wo, bo):
    f32 = np.float32
    hs = np.asarray(hidden_states, f32)
    WqT = np.asarray(Wq, f32).T    # [e_in, e_out]
    WkT = np.asarray(Wk, f32).T
    WvT = np.asarray(Wv, f32).T
    WoT = np.asarray(Wo, f32).T
    bq64 = np.asarray(bq, f32) / 64.0
    bk = np.asarray(bk, f32)
    bv = np.asarray(bv, f32)

    hsT_b = [
        np.ascontiguousarray(hs[b].T).astype(NPBF16) for b in range(B)
    ]
    s_b = [hs[b].sum(axis=0) for b in range(B)]  # [1024] column sums per batch
    in_maps = []
    for i in range(NC):
        g, r = divmod(i, HPC)
        sl = slice(r * EH, (r + 1) * EH)
        wkvt = np.concatenate([WkT[:, sl], WvT[:, sl]], axis=1)
        sk = s_b[g] @ WkT[:, sl]   # colsum of unbiased K0, this core's heads
        sv = s_b[g] @ WvT[:, sl]
        bqt = np.ascontiguousarray(bq64[sl].reshape(2, P).T.astype(np.float32))
        # pack wq/wo into SBUF partition layout for single contiguous DMAs
        wqp = WqT[:, sl].reshape(KC, P, EH).transpose(1, 0, 2).reshape(P, KC * EH)
        wop = WoT[sl, :].reshape(2, P, E).transpose(1, 0, 2).reshape(P, 2 * E)
        in_maps.append({
            "hsT": hsT_b[g],
            "wkvt": np.ascontiguousarray(wkvt).astype(NPBF16),
            "wqp": np.ascontiguousarray(wqp).astype(NPBF16),
            "wop": np.ascontiguousarray(wop).astype(NPBF16),
            "comb0": np.ascontiguousarray(np.concatenate(
                [wkvt[0:P, :], hs[g].T[0:P, 0:T // 2]], axis=1)).astype(NPBF16),
            "bqt": bqt,
            "cbd": cbd_tile(bk[sl], bv[sl], sk, sv),
        })
    return in_maps


def run(inputs, trace=False, **kw):
    """Run on 8 NeuronCores; returns (full_output [B,T,E] f32, BassKernelResults)."""
    nc = _get_nc()
    in_maps = make_in_maps(**inputs)
    # rare transient NRT_EXEC_UNIT_UNRECOVERABLE (~5% of runs observed) —
    # retry up to 2x; the happy path and the compiled NEFF are unchanged
    last_err = None
    for _attempt in range(3):
        try:
            res = run_bass_kernel_spmd(
                nc, in_maps, list(range(NC)), trace=trace, **kw
            )
            break
        except Exception as e:
            last_err = e
    else:
        raise last_err
    bo = np.asarray(inputs["bo"], np.float32)
    out = np.empty((B, T, E), np.float32)
    for g in range(B):
        acc = res.results[g * HPC]["outT"].astype(np.float32)
        for r in range(1, HPC):
            acc = acc + res.results[g * HPC + r]["outT"].astype(np.float32)
        out[g] = acc.T + bo
    return out, res


def kernel(**inputs):
    out, _ = run(inputs, trace=False)
    return out


# revision 60
# speedup vs baseline: 1.1861x; 1.0055x over previous
"""Trainium2 Bass kernel for nn_BartAttention_66786741453241 (8 NeuronCores).

Reference (bugs preserved): no softmax — raw attention scores are used for the
AV matmul, and q is scaled by dh**-0.5 with scores further divided by sqrt(dh),
net 1/dh. The whole computation is therefore LINEAR in V, so we reassociate
    (Q K^T / 64) V  ==  Q (K^T V) / 64
which collapses the [T,T] score matrices into per-head [64,64] K^T V matrices
(~32x fewer attention FLOPs, exact in infinite precision).

Sharding: tensor-parallel by (batch, head-group) — core i handles batch i//4
and heads 4*(i%4) .. 4*(i%4)+4 for ALL 2048 tokens of that batch:
  - fused k|v projection (concatenated weight slice) -> per-head K^T V is
    complete locally: NO collective anywhere,
  - block-diagonal pair tiles of V^T K feed M_j = blockdiag(KTV) @ WoT_pair,
    so the tail is one matmul family: partial out^T = sum_j M_j^T @ qT_j,
  - qT projection for its 4 heads (bias + the net 1/64 scaling folded in),
  - partial out^T (bf16) DMA'd out per core.
The host sums the 4 partials per batch and adds bo — that host-side reduce is
the unshard step for the out_proj input-dim sharding (the "all-reduce after
out_proj" of the standard tensor-parallel recipe).

Schedule (from perfetto iteration; PE stream is packed 216ns/MM warm with
<2us of gaps, [~8us, ~71.5us] of the ~78us NEFF window):
  - input supply is TRANSFER-paced, not trigger-paced: hs rides as 8 2D
    DMAs into ONE [128, 8, 2048] tile — all eight 1024-token FIRST halves
    before any second half — and wkv as 4 two-chunk 3D DMAs interleaved in
    consumption order on the Sync queue.  ~11us of trigger issue for
    ~14.7us of transfer.  wq/wo ride behind hs (needed only from the Q/M
    phases); the tiny bias tiles (bq, cbd) ride the Scalar DGE queue in
    parallel.
  - kv projection waves are emitted CHUNK-major wherever supply is still
    streaming, so the strict PE FIFO order matches DMA arrival order:
    wave 0 = groups 0..6 across psA's 7 banks (hs first halves), then
    group 7 (a->b boundary), wave 1 = groups 8..13 chunk-major over the
    arriving second-half pairs, then groups 14..15 over resident data.
    (A group-major order stalls the FIFO head on a not-yet-arrived chunk
    while ready work sits queued behind it.)
  NOTE: sustained back-to-back benching drives the chip into the P0 power
  state (PE 2.4 -> ~2.0 GHz, kernel ~78us -> ~90us); it recovers after a
  few minutes idle.  The schedule is clock-invariant (gaps stay <2us).
  - k|v biases never touch the device hot path: bias enters V^T K as
    rank-1 terms sv*bk^T + bv*sk^T + T*bv*bk^T, host-computed from hs
    column sums into per-pair diagonal [64,64] blocks (cbd) and added in
    the single V^T K evacuation.  kv evacuations are plain copies
    alternating Vector/Scalar.
  - V^T K matmuls (N=128, LDWEIGHTS-bound as a phase) are emitted right
    behind each group's evacuation, so their weight loads hide under
    neighbouring N=512 matmuls.
  - PE warm-up: 7 dummy matmuls bridge queue-start to first-chunk arrival
    and engage the HAM clock un-throttle (2.4 GHz after ~3.4us sustained).
  - tail: the last out^T chunk drains as [0:1024] after tg1, [1024:1536]
    after tg2 (computed as parallel-evacuated N=256 half-groups), and a
    final [1536:2048] whose two half-group evacuations AND DMA trigger all
    sit program-ordered on the Scalar queue — no cross-engine semaphore
    hop after the last matmul.
  - all matmuls bf16 (fp32 PSUM accumulate); end-to-end relative error vs
    the f32 reference ~4.7e-3 (gate 2e-2).
Rejected after measurement: a Gram-matrix reformulation (G = hs^T hs) needs
an inter-core AllReduce, but a 0.5 MB 8-core AllReduce measures ~70us warm on
this rig — any collective dwarfs the savings, so the collective-free
(batch x head-group) sharding with host-side partial summation stands.
"""

import os
import sys
import types

import numpy as np
import ml_dtypes

import concourse.bacc as bacc
import concourse.mybir as mybir
import concourse.tile as tile
from concourse.bass_utils import run_bass_kernel_spmd

BF16 = mybir.dt.bfloat16
F32 = mybir.dt.float32
NPBF16 = ml_dtypes.bfloat16

E = 1024        # embed dim
H = 16          # heads
DH = 64         # head dim
B, T = 2, 2048
NC = 8          # cores
P = 128
KC = E // P     # 8 contraction chunks for the in-projections
HPC = 4         # heads per core
EH = HPC * DH   # 256: per-core q/k/v feature width
TG = T // 512   # 4 moving-dim groups of 512 tokens
TTC = T // P    # 16 token chunks per core
Ident = mybir.ActivationFunctionType.Identity
N_DUMMY = 8     # PE warm-up matmuls (N=512)


def _install_axon_profile_hook():
    """Make trace=True usable under axon: register the NTFF hook that the
    staged antenv lacks, and neuter artifact upload (no bucket here). Safe
    no-op when pieces are missing."""
    try:
        import concourse.bass_utils as bu
        bu.upload_artifacts = lambda tmpdir: "local://" + tmpdir
    except Exception:
        pass
    if "antenv.axon_hooks" in sys.modules:
        return
    hook = None
    try:
        from trn_agent_boot.trn_boot import _ntff_profile_via_ctypes
        so = "/opt/axon/libaxon_pjrt.so"
        if os.path.exists(so):
            hook = _ntff_profile_via_ctypes(so)
    except Exception:
        hook = None
    mod = types.ModuleType("antenv.axon_hooks")
    mod.get_axon_ntff_profile_hook = lambda: hook
    mod.set_axon_ntff_profile_hook = lambda h: None
    sys.modules["antenv.axon_hooks"] = mod


def build():
    """Build + compile the per-core SPMD graph (identical on all 8 cores)."""
    nc = bacc.Bacc("TRN2", target_bir_lowering=False, debug=False, num_devices=NC)

    hsT = nc.dram_tensor("hsT", [E, T], BF16, kind="ExternalInput")       # 4 MB
    # every wkv chunk c shares partition rows (E-rows of chunk c) with hs
    # chunk c's first token half: host-fused so ONE trigger delivers one
    # chunk-major septet's complete input (weights + tokens) atomically
    combT = nc.dram_tensor("combT", [P, KC, 2 * EH + T // 2], BF16, kind="ExternalInput")
    # wq/wo pre-packed on host into SBUF partition layout -> one contiguous
    # 2D DMA each (128 descriptors, ~0.6us trigger) instead of a 1024-
    # descriptor 3D rearrange (~2.4us trigger)
    wqp = nc.dram_tensor("wqp", [P, KC * EH], BF16, kind="ExternalInput")  # 0.5 MB
    wop = nc.dram_tensor("wop", [P, 2 * E], BF16, kind="ExternalInput")    # 0.5 MB
    # bq/64 per-partition columns; cbd = host-computed k|v-bias correction to
    # the block-diagonal V^T K pair tiles (bias enters V^T K as rank-1 terms
    # sv*bk^T + bv*sk^T + T*bv*bk^T, computable on host from column sums of
    # hs — so K/V are projected WITHOUT bias and evacs are plain copies)
    bqt = nc.dram_tensor("bqt", [P, 2], F32, kind="ExternalInput")
    cbd = nc.dram_tensor("cbd", [P, 2, P], BF16, kind="ExternalInput")
    outT = nc.dram_tensor("outT", [E, T], BF16, kind="ExternalOutput")

    with tile.TileContext(nc) as tc:
        with (
            tc.tile_pool(name="sb", bufs=1) as sb,
            tc.tile_pool(name="stg", bufs=3) as stg,
            tc.tile_pool(name="psA", bufs=7, space="PSUM") as psA,
            tc.tile_pool(name="psB", bufs=1, space="PSUM") as psB,
        ):
            # ---- PE warm-up: dummy matmuls on a memset tile keep the PE's
            # HAM activity window busy during the input-DMA wait so the real
            # stream starts at 2.4 GHz instead of ramping from 1.2.
            dum_w = sb.tile([P, P], BF16, tag="dum_w")
            nc.gpsimd.memset(dum_w[:], 0.0)
            dum_x = sb.tile([P, 512], BF16, tag="dum_x")
            nc.gpsimd.memset(dum_x[:], 0.0)
            dum_ps = psB.tile([P, 512], F32, tag="psB")
            for _ in range(N_DUMMY):
                nc.tensor.matmul(dum_ps[:], dum_w[:], dum_x[:], start=True, stop=True)
            dum_out = sb.tile([P, 4], BF16, tag="dum_out")
            nc.vector.tensor_copy(dum_out[:], dum_ps[:, 0:4])

            # ---- input loads, all on the Sync queue in consumption byte
            # order.  The kv bias rides as a 1 KB [1,512] row (broadcast
            # across partitions on-device via a K=1 outer-product matmul)
            # and bq/64 as a tiny [128,2] tile, so no 0.26 MB bias tile
            # pollutes the critical byte path.  hs is ONE [128, 8, 2048]
            # tile; its FIRST token halves (tokens 0:1024, enough to finish
            # kv groups 0..7) stream before any second half, so the PE
            # saturates with backlog instead of chasing chunk arrivals.
            hs_big = sb.tile([P, KC, T], BF16, tag="hs")
            bq_sb = sb.tile([P, 2], F32, tag="bq")
            hs3 = hsT.ap().rearrange("(c p) t -> p c t", p=P)

            TH = T // 2  # 1024-token halves
            comb_big = sb.tile([P, KC, 2 * EH + TH], BF16, tag="comb")

            def wkv_ap(c):
                return comb_big[:, c, 0:2 * EH]

            def hs_ap(c, lo, hi):
                if hi <= TH:
                    return comb_big[:, c, 2 * EH + lo:2 * EH + hi]
                return hs_big[:, c, lo:hi]

            def d_hs_b(c0, c1):
                nc.sync.dma_start(hs_big[:, c0:c1, TH:T], hs3[:, c0:c1, TH:T])

            # bias tiles ride the Scalar DGE queue: issue in parallel with
            # Sync's critical wkv/hs triggers, ~66 KB so no HBM contention.
            nc.scalar.dma_start(bq_sb[:], bqt[:, :])
            cbd_sb = sb.tile([P, 2, P], BF16, tag="cbd")
            nc.scalar.dma_start(cbd_sb[:], cbd[:, :, :])
            for c in range(KC):
                nc.sync.dma_start(comb_big[:, c, :], combT[:, c, :])
            d_hs_b(0, 2)
            d_hs_b(2, 4)
            d_hs_b(4, 6)
            d_hs_b(6, 8)

            # wq/wo ride the Sync queue behind all hs bytes (host-packed ->
            # cheap 128-descriptor 2D triggers)
            wq_big = sb.tile([P, KC * EH], BF16, tag="wq")
            nc.sync.dma_start(wq_big[:], wqp[:, :])
            wo_sb = sb.tile([P, 2, E], BF16, tag="wo")
            nc.sync.dma_start(wo_sb[:].rearrange("p c e -> p (c e)"), wop[:, :])

            # ---- fused k|v projection: [128 tokens, k(4 heads)|v(4 heads)]
            # with the pair-packed V^T K matmuls emitted right behind each
            # group's evacuation (their LDWEIGHTS hide under N=512 matmuls).
            # vtk_ps takes the psB bank for the whole loop (free once the
            # dummies evacuate); kv groups rotate through psA's 7 banks.
            kv_sb = [
                sb.tile([P, 2 * EH], BF16, tag=f"kv{tt}", name=f"kv{tt}")
                for tt in range(TTC)
            ]
            # per-head K^T V (pair-packed): ONE MM per (pair, chunk) —
            # lhsT = [v_A|v_B] against rhs = [k_A|k_B]; both pairs'
            # [128,128] outputs share one PSUM bank (256 f32/partition).
            vtk_ps = psB.tile([P, 2 * P], F32, tag="psB")

            def kv_evac(tt, ps):
                # plain copy (no bias — host folds it into cbd), alternating
                # engines so evacuations don't serialize on one engine
                if tt % 2 == 0:
                    nc.vector.tensor_copy(kv_sb[tt][:], ps[:])
                else:
                    nc.scalar.copy(kv_sb[tt][:], ps[:])

            def vtk_mm(tt):
                for j in range(HPC // 2):
                    # start=True clears has_written for the WHOLE bank, so
                    # only the very first matmul may carry it: pair 1's
                    # first write then lands on cleared bits -> overwrite,
                    # which is exactly the accumulation restart we want.
                    nc.tensor.matmul(
                        vtk_ps[:, j * P:(j + 1) * P],
                        kv_sb[tt][:, EH + 2 * j * DH:EH + (2 * j + 2) * DH],
                        kv_sb[tt][:, 2 * j * DH:(2 * j + 2) * DH],
                        start=(tt == 0 and j == 0),
                        stop=(tt == TTC - 1 and j == HPC // 2 - 1),
                    )

            # wave 0: token groups 0..6 CHUNK-major across psA's 7 banks, so
            # the PE FIFO order matches the hs half-chunk arrival order (a
            # token-group-major order would stall the FIFO head on chunk c
            # while ready work for earlier chunks sits queued behind it)
            NW0 = 7
            ps_w = [
                psA.tile([P, 512], F32, tag="psA", name=f"kvps{tt}")
                for tt in range(NW0)
            ]
            for c in range(KC):
                for tt in range(NW0):
                    nc.tensor.matmul(
                        ps_w[tt][:],
                        hs_ap(c, tt * P, (tt + 1) * P),
                        wkv_ap(c),
                        start=(c == 0),
                        stop=(c == KC - 1),
                    )
            for tt in range(NW0):
                kv_evac(tt, ps_w[tt])
                vtk_mm(tt)

            # group 7 (tokens 896:1024, still in the a-halves) group-major —
            # it bridges the a->b supply boundary
            def kv_group(tt):
                ps = psA.tile([P, 512], F32, tag="psA", name=f"kvg{tt}")
                for c in range(KC):
                    nc.tensor.matmul(
                        ps[:],
                        hs_ap(c, tt * P, (tt + 1) * P),
                        wkv_ap(c),
                        start=(c == 0),
                        stop=(c == KC - 1),
                    )
                kv_evac(tt, ps)
                vtk_mm(tt)

            kv_group(7)
            # wave 1: groups 8..13 CHUNK-major again — their tokens live in
            # the hs second halves, which arrive as chunk PAIRS while this
            # wave runs; chunk-major keeps the PE FIFO aligned with arrival
            # order (group-major would block the FIFO head on the last
            # b-pair while ready work for earlier chunks waits behind it)
            ps_w2 = [
                psA.tile([P, 512], F32, tag="psA", name=f"kvps2{tt}")
                for tt in range(8, 14)
            ]
            for c in range(KC):
                for i, tt in enumerate(range(8, 14)):
                    nc.tensor.matmul(
                        ps_w2[i][:],
                        hs_ap(c, tt * P, (tt + 1) * P),
                        wkv_ap(c),
                        start=(c == 0),
                        stop=(c == KC - 1),
                    )
            for i, tt in enumerate(range(8, 14)):
                kv_evac(tt, ps_w2[i])
                vtk_mm(tt)
            # groups 14..15 group-major over resident data
            kv_group(14)
            kv_group(15)

            # ---- block-diagonal V^T K pair tiles (useful diagonal [64,64]
            # blocks; VTK_h = KTV_h^T), then fold the out-projection:
            # M_j = blockdiag(KTV_A, KTV_B) @ WoT_pair
            vtk_bd = [
                sb.tile([P, P], BF16, tag=f"vtk_bd{j}", name=f"vtk_bd{j}")
                for j in range(HPC // 2)
            ]
            for j in range(HPC // 2):
                nc.gpsimd.memset(vtk_bd[j][:], 0.0)
            for j in range(HPC // 2):
                nc.vector.tensor_add(
                    vtk_bd[j][0:DH, 0:DH], vtk_ps[0:DH, j * P:j * P + DH],
                    cbd_sb[0:DH, j, 0:DH],
                )
                nc.vector.tensor_add(
                    vtk_bd[j][DH:2 * DH, DH:2 * DH],
                    vtk_ps[DH:2 * DH, j * P + DH:(j + 1) * P],
                    cbd_sb[DH:2 * DH, j, DH:2 * DH],
                )
            m_sb = [
                sb.tile([P, E], BF16, tag=f"m{j}", name=f"m{j}")
                for j in range(HPC // 2)
            ]
            for j in range(HPC // 2):
                for half in range(2):
                    ps = psB.tile([P, 512], F32, tag="psB")
                    nc.tensor.matmul(
                        ps[:],
                        vtk_bd[j][:],
                        wo_sb[:, j, half * 512:(half + 1) * 512],
                        start=True,
                        stop=True,
                    )
                    nc.vector.tensor_copy(
                        m_sb[j][:, half * 512:(half + 1) * 512], ps[:]
                    )

            # ---- qT projection [e_out 256, tokens], bias + 1/64 folded
            q_sb = [
                sb.tile([P, T], BF16, tag=f"q{m}", name=f"q{m}")
                for m in range(EH // P)
            ]
            for m in range(EH // P):
                for tg in range(TG):
                    ps = psA.tile([P, 512], F32, tag="psA")
                    for c in range(KC):
                        nc.tensor.matmul(
                            ps[:],
                            wq_big[:, c * EH + m * P:c * EH + (m + 1) * P],
                            hs_ap(c, tg * 512, (tg + 1) * 512),
                            start=(c == 0),
                            stop=(c == KC - 1),
                        )
                    nc.scalar.activation(
                        q_sb[m][:, tg * 512:(tg + 1) * 512], ps[:], Ident,
                        bias=bq_sb[:, m:m + 1], scale=1.0 / 64.0,
                    )

            # ---- partial out^T = sum_j M_j^T @ qT_j (no bias: host adds bo)
            for m in range(KC):
                o_stage = stg.tile([P, T], BF16, tag="ostg")
                last = m == KC - 1
                for tg in range(TG):
                    if last and tg >= 2:
                        # last chunk's tg2+tg3 as N=256 half-groups in
                        # separate banks: Vector/Scalar evacuate in
                        # parallel, the DMA pieces chain early, and the
                        # very last piece is only 64 KB.
                        ph = [
                            psA.tile([P, 256], F32, tag="psA", name=f"ph{tg}{h}")
                            for h in range(2)
                        ]
                        for h in range(2):
                            for c in range(2):
                                nc.tensor.matmul(
                                    ph[h][:],
                                    m_sb[c][:, m * P:(m + 1) * P],
                                    q_sb[c][:, tg * 512 + h * 256:
                                             tg * 512 + (h + 1) * 256],
                                    start=(c == 0),
                                    stop=(c == 1),
                                )
                        lo = tg * 512
                        if tg == 2:
                            nc.vector.tensor_copy(
                                o_stage[:, lo:lo + 256], ph[0][:]
                            )
                            nc.scalar.copy(o_stage[:, lo + 256:lo + 512], ph[1][:])
                            nc.sync.dma_start(
                                outT[m * P:(m + 1) * P, 1024:1536],
                                o_stage[:, 1024:1536],
                            )
                        else:
                            # final 512 tokens: Vector (idle by now) takes
                            # the first half in parallel with Scalar's
                            # second; Scalar then triggers the single last
                            # DMA program-ordered behind its own copy, with
                            # only one cross-engine semaphore already long
                            # satisfied.
                            nc.vector.tensor_copy(o_stage[:, lo:lo + 256], ph[0][:])
                            nc.scalar.copy(o_stage[:, lo + 256:lo + 512], ph[1][:])
                            nc.scalar.dma_start(
                                outT[m * P:(m + 1) * P, 1536:T],
                                o_stage[:, 1536:T],
                            )
                        continue
                    ps = psA.tile([P, 512], F32, tag="psA")
                    for c in range(2):
                        nc.tensor.matmul(
                            ps[:],
                            m_sb[c][:, m * P:(m + 1) * P],
                            q_sb[c][:, tg * 512:(tg + 1) * 512],
                            start=(c == 0),
                            stop=(c == 1),
                        )
                    if tg % 2 == 0:
                        nc.vector.tensor_copy(o_stage[:, tg * 512:(tg + 1) * 512], ps[:])
                    else:
                        nc.scalar.copy(o_stage[:, tg * 512:(tg + 1) * 512], ps[:])
                    if last and tg == 1:
                        # drain the last chunk eagerly so only the tail
                        # pieces remain after the final evacuations
                        nc.sync.dma_start(
                            outT[m * P:(m + 1) * P, 0:1024], o_stage[:, 0:1024]
                        )
                if not last:
                    nc.sync.dma_start(outT[m * P:(m + 1) * P, :], o_stage[:])

    nc.compile()
    return nc


_NC_CACHE = None


def _get_nc():
    global _NC_CACHE
    if _NC_CACHE is None:
        _install_axon_profile_hook()
        _NC_CACHE = build()
    return _NC_CACHE


def comb_host(wkvt, hs_g):
    """Fuse per-chunk wkv weights with hs first token halves: [128, 8, 1536],
    [:, c, 0:512] = wkvt rows c*128..(c+1)*128, [:, c, 512:] = hsT same rows,
    tokens 0:1024."""
    t = np.empty((P, KC, 2 * EH + T // 2), np.float32)
    hsT_g = hs_g.T  # [E, T]
    for c in range(KC):
        t[:, c, 0:2 * EH] = wkvt[c * P:(c + 1) * P, :]
        t[:, c, 2 * EH:] = hsT_g[c * P:(c + 1) * P, 0:T // 2]
    return np.ascontiguousarray(t).astype(NPBF16)


def cbd_tile(bk_sl, bv_sl, sk, sv):
    """Host-side k|v-bias correction to the block-diagonal V^T K pair tiles.
    V^T K = V0^T K0 + sv*bk^T + bv*sk^T + T*bv*bk^T   (per head), where
    sk/sv are the column sums of the UNbiased K0/V0 = Wk/Wv @ hs.sum(tokens).
    Layout [128, 2, 128]: pair j's heads (2j, 2j+1) on the two diagonal
    [64,64] blocks of [:, j, :]."""
    t = np.zeros((P, 2, P), np.float32)
    for j in range(HPC // 2):
        for hh in range(2):
            h = 2 * j + hh
            sl = slice(h * DH, (h + 1) * DH)
            r = slice(hh * DH, (hh + 1) * DH)
            c = np.outer(sv[sl], bk_sl[sl]) + np.outer(bv_sl[sl], sk[sl]) \
                + T * np.outer(bv_sl[sl], bk_sl[sl])
            t[r, j, r] = c
    return t.astype(NPBF16)


def make_in_maps(hidden_states, Wq, bq, Wk, bk, Wv, bv, Wo, bo):
    f32 = np.float32
    hs = np.asarray(hidden_states, f32)
    WqT = np.asarray(Wq, f32).T    # [e_in, e_out]
    WkT = np.asarray(Wk, f32).T
    WvT = np.asarray(Wv, f32).T
    WoT = np.asarray(Wo, f32).T
    bq64 = np.asarray(bq, f32) / 64.0
    bk = np.asarray(bk, f32)
    bv = np.asarray(bv, f32)

    hsT_b = [
        np.ascontiguousarray(hs[b].T).astype(NPBF16) for b in range(B)
    ]
    s_b = [hs[b].sum(axis=0) for b in range(B)]  # [1024] column sums per batch
    in_maps = []
    for i in range(NC):
        g, r = divmod(i, HPC)
        sl = slice(r * EH, (r + 1) * EH)
        wkvt = np.concatenate([WkT[:, sl], WvT[:, sl]], axis=1)
        sk = s_b[g] @ WkT[:, sl]   # colsum of unbiased K0, this core's heads
        sv = s_b[g] @ WvT[:, sl]
        bqt = np.ascontiguousarray(bq64[sl].reshape(2, P).T.astype(np.float32))
        # pack wq/wo into SBUF partition layout for single contiguous DMAs
        wqp = WqT[:, sl].reshape(KC, P, EH).transpose(1, 0, 2).reshape(P, KC * EH)
        wop = WoT[sl, :].reshape(2, P, E).transpose(1, 0, 2).reshape(P, 2 * E)
        in_maps.append({
            "hsT": hsT_b[g],

            "wqp": np.ascontiguousarray(wqp).astype(NPBF16),
            "wop": np.ascontiguousarray(wop).astype(NPBF16),
            "combT": comb_host(wkvt, hs[g]),
            "bqt": bqt,
            "cbd": cbd_tile(bk[sl], bv[sl], sk, sv),
        })
    return in_maps


def run(inputs, trace=False, **kw):
    """Run on 8 NeuronCores; returns (full_output [B,T,E] f32, BassKernelResults)."""
    nc = _get_nc()
    in_maps = make_in_maps(**inputs)
    # rare transient NRT_EXEC_UNIT_UNRECOVERABLE (~5% of runs observed) —
    # retry up to 2x; the happy path and the compiled NEFF are unchanged
    last_err = None
    for _attempt in range(3):
        try:
            res = run_bass_kernel_spmd(
                nc, in_maps, list(range(NC)), trace=trace, **kw
            )
            break
        except Exception as e:
            last_err = e
    else:
        raise last_err
    bo = np.asarray(inputs["bo"], np.float32)
    out = np.empty((B, T, E), np.float32)
    for g in range(B):
        acc = res.results[g * HPC]["outT"].astype(np.float32)
        for r in range(1, HPC):
            acc = acc + res.results[g * HPC + r]["outT"].astype(np.float32)
        out[g] = acc.T + bo
    return out, res


def kernel(**inputs):
    out, _ = run(inputs, trace=False)
    return out
